# revision 1
# baseline (speedup 1.0000x reference)
"""TRN2 Bass kernel for nn_Block1_43542378447225.

Pipeline (per sample, one NeuronCore; batch=2 -> cores 0/1 do real work):
  conv1 -> relu -> conv2 -> relu -> Hopfield(z2) -> D
  backward (w2b matmul, mask, Scomb matmul) -> C  (e_sum in composite-window form)
  blocked e_min via permutation matmuls -> mask -> masked patch forward -> z2_masked
  Hopfield(z2_masked) -> output

Host precomputes im2col patches (P1 [48,256], X [100,192]) and weight layouts,
packed into few DMA-able blocks; the device does all matmuls/softmax/masking.

Layout conventions:
  pq = p*8+q (64 output positions), uv = u*10+v (100 composite-window offsets)
  chunk t = conv2 kernel row kr, a = conv2 kernel col ks
  kc = t*128 + a*32 + c1 (hidden index, 4 chunks of 128 partitions)
"""
import numpy as np

import concourse.bass as bass
import concourse.bacc as bacc
import concourse.mybir as mybir
import concourse.tile as tile
from concourse.tile import add_dep_helper
from concourse.bass_utils import run_bass_kernel_spmd

F32 = mybir.dt.float32
BF16 = mybir.dt.bfloat16
AF = mybir.ActivationFunctionType
ALU = mybir.AluOpType

N_CORES = 8
BETA = 0.125  # 1/sqrt(64)
BIG = 1.0e30

_CACHE = {}


# ---------------------------------------------------------------- host prep
def _build_scomb_w1big(w1):
    w1s = w1.sum(axis=1)
    Scomb = np.zeros((4, 32, 4, 100), np.float32)  # [a, c1, t, uv]
    W1big = np.zeros((100, 3, 4, 4, 32), np.float32)  # [uv, h, t, a, c1]
    for t in range(4):
        for a in range(4):
            for u in range(10):
                ki = u - 2 * t
                if not (0 <= ki < 4):
                    continue
                for v in range(10):
                    kj = v - 2 * a
                    if not (0 <= kj < 4):
                        continue
                    Scomb[a, :, t, u * 10 + v] = w1s[:, ki, kj]
                    W1big[u * 10 + v, :, t, a, :] = w1[:, :, ki, kj].T
    # partition index = a*32+c1 -> merge (a, c1); free = t*100+uv
    Scomb = Scomb.reshape(128, 400)
    W1big = W1big.reshape(100, 1536)
    return Scomb, W1big


def _host_prep(w1, b1, w2, b2, K, Vw):
    # wA [128, 897]: w2fT | b2 | KT | Vw | ident
    wA = np.zeros((128, 897), np.float32)
    wA[:, 0:256] = np.transpose(w2, (3, 1, 2, 0)).reshape(128, 256)  # w2fT
    wA[0:64, 256:257] = b2[:, None]
    wA[0:64, 257:769] = K.T
    wA[0:64, 769:833] = Vw
    wA[0:64, 833:897] = np.eye(64, dtype=np.float32)
    wA[64:128, 833:897] = np.eye(64, dtype=np.float32)

    Scomb, W1big = _build_scomb_w1big(w1)
    PermF = np.zeros((100, 9, 16), np.float32)
    for k in range(9):
        dp, dq = k // 3 - 1, k % 3 - 1
        for im in range(4):
            u = 4 * dp + im + 3
            if not (0 <= u < 10):
                continue
            for jm in range(4):
                v = 4 * dq + jm + 3
                if not (0 <= v < 10):
                    continue
                PermF[u * 10 + v, k, im * 4 + jm] = 1.0
    CandM = np.zeros((100, 3, 128), np.float32)
    for k in range(9):
        cc, kk = divmod(k, 4)
        CandM[:, cc, kk * 32:kk * 32 + 16] = PermF[:, k, :]
    PermB = np.transpose(PermF, (2, 1, 0)).reshape(16, 900)

    # wB [128, 2196]: w2b | Scomb | CandM_ext | PermB
    wB = np.zeros((128, 2196), np.float32)
    wB[0:64, 0:512] = 2.0 * np.transpose(w2, (0, 2, 3, 1)).reshape(64, 512)
    wB[:, 512:912] = Scomb
    wB[0:100, 912:1296] = CandM.reshape(100, 384)
    wB[0:16, 1296:2196] = PermB

    # wC [128, 1536]: W1big rows 0:100
    wC = np.zeros((128, 1536), np.float32)
    wC[0:100, :] = W1big

    return {"wA": wA, "wB": wB, "wC": wC,
            "_w1f": np.ascontiguousarray(np.transpose(w1, (2, 3, 1, 0)).reshape(48, 32)),
            "_b1": np.ascontiguousarray(b1[:, None])}


def _sample_prep(x_s, w1f, b1c):
    xp1 = np.pad(x_s, ((0, 0), (1, 1), (1, 1)))
    xp3 = np.pad(x_s, ((0, 0), (3, 3), (3, 3)))
    P1 = np.zeros((4, 4, 3, 16, 16), np.float32)
    for kr in range(4):
        for ks in range(4):
            P1[kr, ks] = xp1[:, kr:kr + 32:2, ks:ks + 32:2][:, :16, :16]
    X = np.zeros((10, 10, 3, 8, 8), np.float32)
    for u in range(10):
        for v in range(10):
            X[u, v] = xp3[:, u:u + 32:4, v:v + 32:4][:, :8, :8]
    cv = np.zeros((48, 289), np.float32)
    cv[:, 0:256] = P1.reshape(48, 256)
    cv[:, 256:288] = w1f
    cv[0:32, 288:289] = b1c
    return cv, X.reshape(100, 192).copy()


# ---------------------------------------------------------------- device build
def _hopfield(nc, sb, ps, z_sb, KT_bf, KV_sb, ident_sb, tag):
    """z_sb [64(c), 64(pq)] -> returns q_ps [64(c), 64(pq)] PSUM tile.
    Scores bounded (|beta*S| small) -> softmax skips max subtraction.
    The 512 codebook scores sit as two 64-row halves in 128 partitions so
    exp/normalize run at full lane width."""
    z_bf = sb.tile([64, 64], BF16, tag=f"zbf{tag}", name=f"zbf{tag}")
    nc.vector.tensor_copy(out=z_bf[:], in_=z_sb)
    S_ps = ps.tile([128, 256], F32, tag="S", bufs=1, name=f"S{tag}")
    nc.tensor.matmul(S_ps[0:64, :], z_bf[:], KT_bf[:, 0:256],
                     start=True, stop=True)
    nc.tensor.matmul(S_ps[64:128, :], z_bf[:], KT_bf[:, 256:512],
                     start=True, stop=True)
    att = sb.tile([128, 256], F32, tag=f"att{tag}", name=f"att{tag}")
    ssum = sb.tile([128, 1], F32, tag=f"ssum{tag}", name=f"ssum{tag}")
    nc.scalar.activation(out=att[:], in_=S_ps[:], func=AF.Exp,
                         bias=0.0, scale=BETA, accum_out=ssum[:])
    shi = sb.tile([64, 1], F32, tag=f"shi{tag}", name=f"shi{tag}")
    nc.vector.tensor_copy(out=shi[:], in_=ssum[64:128, :])
    stot = sb.tile([64, 1], F32, tag=f"stot{tag}", name=f"stot{tag}")
    nc.vector.tensor_tensor(out=stot[:], in0=ssum[0:64, :], in1=shi[:],
                            op=ALU.add)
    rec = sb.tile([128, 1], F32, tag=f"rec{tag}", name=f"rec{tag}")
    nc.vector.reciprocal(rec[0:64, :], stot[:])
    nc.vector.reciprocal(rec[64:128, :], stot[:])
    nc.vector.tensor_scalar_mul(att[:], att[:], rec[:])
    attT = sb.tile([128, 4, 64], F32, tag=f"attT{tag}", name=f"attT{tag}")
    for t in range(4):
        half, col = divmod(t, 2)
        tr_ps = ps.tile([128, 64], F32, tag="g128", bufs=4, name=f"tr{tag}{t}")
        nc.tensor.transpose(
            tr_ps[:], att[half * 64:half * 64 + 64, col * 128:(col + 1) * 128],
            ident_sb[half * 64:half * 64 + 64, :])
        if t % 2 == 0:
            nc.vector.tensor_copy(out=attT[:, t, :], in_=tr_ps[:])
        else:
            nc.scalar.copy(out=attT[:, t, :], in_=tr_ps[:])
    q_ps = ps.tile([64, 64], F32, tag="q64", bufs=2, name=f"q{tag}")
    for t in range(4):
        half, col = divmod(t, 2)
        m0 = half * 256 + col * 128
        nc.tensor.matmul(q_ps[:], KV_sb[:, t, :], attT[:, t, :],
                         start=(t == 0), stop=(t == 3))
    return q_ps


def _build_nc(debug=False):
    nc = bacc.Bacc("TRN2", target_bir_lowering=False, debug=False,
                   num_devices=N_CORES)
    d_cv = nc.dram_tensor("cv", [48, 289], F32, kind="ExternalInput")
    d_smpl = nc.dram_tensor("smpl", [100, 192], F32, kind="ExternalInput")
    d_wA = nc.dram_tensor("wA", [128, 897], F32, kind="ExternalInput")
    d_wB = nc.dram_tensor("wB", [128, 2196], F32, kind="ExternalInput")
    d_wC = nc.dram_tensor("wC", [128, 1536], F32, kind="ExternalInput")
    out_t = nc.dram_tensor("out", [64, 64], F32, kind="ExternalOutput")
    probes = {}

    def probe(name, shape):
        if debug:
            probes[name] = nc.dram_tensor("probe_" + name, shape, F32,
                                          kind="ExternalOutput")
        return probes.get(name)

    with tile.TileContext(nc) as tc:
        with tc.tile_pool(name="sb", bufs=1) as sb, \
             tc.tile_pool(name="ps", bufs=1, space="PSUM") as ps:
            # ---- PE warm-up: tiny matmuls on memset data keep the PE
            # pipeline out of its cold p-state before conv1 arrives.
            warm = sb.tile([2, 8], F32, tag="warm")
            nc.vector.memset(warm[:], 0.0)
            for w_ in range(3):
                warm_ps = ps.tile([8, 8], F32, tag="q64", bufs=2,
                                  name=f"warm{w_}")
                nc.tensor.matmul(warm_ps[:], warm[0:2, :], warm[0:2, :],
                                 start=True, stop=True)

            # ---- loads: 2 HWDGE queues (SP: cv+smpl+wB, ACT: wA+wC)
            cv = sb.tile([48, 289], F32, tag="cv")
            nc.sync.dma_start(out=cv[:], in_=d_cv[:])
            wA = sb.tile([128, 897], F32, tag="wA")
            nc.scalar.dma_start(out=wA[:], in_=d_wA[:])
            smpl = sb.tile([100, 192], F32, tag="smpl")
            nc.sync.dma_start(out=smpl[:], in_=d_smpl[:])
            wB = sb.tile([128, 2196], F32, tag="wB")
            nc.sync.dma_start(out=wB[:], in_=d_wB[:])
            wC = sb.tile([128, 1536], F32, tag="wC")
            nc.scalar.dma_start(out=wC[:], in_=d_wC[:])
            C_ext = sb.tile([100, 64], F32, tag="C_ext")

            P1 = cv[:, 0:256]
            X = smpl[:].rearrange("u (h q) -> u h q", h=3)
            w1f = cv[:, 256:288]
            b1 = cv[0:32, 288:289]
            w2fT = wA[:, 0:256].rearrange("k (t c) -> k t c", t=4)
            b2 = wA[0:64, 256:257]
            KT = wA[0:64, 257:769]
            Vw = wA[0:64, 769:833]
            ident = wA[:, 833:897]
            w2b = wB[0:64, 0:512]
            Scomb = wB[:, 512:912].rearrange("k (t u) -> k t u", t=4)
            CandM = wB[0:100, 912:1296].rearrange("u (c k) -> u c k", c=3)
            PermB = wB[0:16, 1296:2196]
            W1big = wC[0:100, :].rearrange("u (h t k) -> u h t k", h=3, t=4)

            # ---- conv1 + relu into padded a1p [32, 18, 18]
            a1_ps = ps.tile([32, 256], F32, tag="a1", bufs=1)
            nc.tensor.matmul(a1_ps[:], w1f, P1, start=True, stop=True)
            a1p = sb.tile([32, 18, 18], F32, tag="a1p")
            nc.vector.memset(a1p[:], 0.0)
            nc.scalar.activation(
                out=a1p[:, 1:17, 1:17],
                in_=a1_ps[:].rearrange("c (p q) -> c p q", p=16),
                func=AF.Relu, bias=b1, scale=1.0)

            # ---- P2 im2col: P2[a*32+c1, t, p, q] = a1p[c1, 2p+t, 2q+a]
            P2 = sb.tile([128, 4, 8, 8], F32, tag="P2")
            a1p_ap = a1p[:]
            p2_last = None
            for a in range(4):
                src = bass.AP(
                    tensor=a1p_ap.tensor,
                    offset=a1p_ap.offset + a,
                    ap=[[324, 32], [18, 4], [36, 8], [2, 8]])
                eng = nc.vector if a < 3 else nc.gpsimd
                inst = eng.tensor_copy(out=P2[a * 32:(a + 1) * 32, :, :, :],
                                       in_=src)
                if a == 2:
                    p2_last = inst
            P2f = P2[:].rearrange("k t p q -> k t (p q)")
            M1W = sb.tile([128, 4, 64], F32, tag="M1W")
            nc.vector.tensor_scalar(out=M1W[:], in0=P2f, scalar1=0.0,
                                    scalar2=None, op0=ALU.not_equal)

            # ---- conv2 + relu -> z2 [64, 64], m2
            z2_ps = ps.tile([64, 64], F32, tag="q64", bufs=2)
            for t in range(4):
                conv2_last = nc.tensor.matmul(z2_ps[:], w2fT[:, t, :],
                                              P2f[:, t, :],
                                              start=(t == 0), stop=(t == 3))
            z2 = sb.tile([64, 64], F32, tag="z2")
            z2_relu = nc.scalar.activation(out=z2[:], in_=z2_ps[:],
                                           func=AF.Relu, bias=b2, scale=1.0)
            if debug:
                nc.sync.dma_start(out=probe("z2", [64, 64])[:], in_=z2[:])

            KT_bf = sb.tile([64, 512], BF16, tag="KT_bf")
            ktbf_inst = nc.vector.tensor_copy(out=KT_bf[:], in_=KT)
            add_dep_helper(p2_last.ins, ktbf_inst.ins, sync=False,
                           reason="P2 gather gates conv2; KT_bf can wait")
            # ---- KV = K @ Vw chunked [128, 4, 64]
            KV = sb.tile([128, 4, 64], F32, tag="KV")
            for t in range(4):
                kv_ps = ps.tile([128, 64], F32, tag="g128", bufs=4,
                                name=f"kv{t}")
                kvmm = nc.tensor.matmul(kv_ps[:], KT[:, t * 128:(t + 1) * 128],
                                        Vw, start=True, stop=True)
                kvcp = nc.scalar.copy(out=KV[:, t, :], in_=kv_ps[:])
                if t == 0:
                    add_dep_helper(conv2_last.ins, kvmm.ins, sync=False,
                                   reason="conv2 on the critical path")
                    add_dep_helper(z2_relu.ins, kvcp.ins, sync=False,
                                   reason="z2 relu on the critical path")

            # ---- Hopfield #1 -> D*m2 (factor 2 folded into w2b)
            q_ps = _hopfield(nc, sb, ps, z2[:], KT_bf[:], KV, ident, "1")
            m2 = sb.tile([64, 64], F32, tag="m2")
            nc.vector.tensor_scalar(out=m2[:], in0=z2[:], scalar1=0.0,
                                    scalar2=None, op0=ALU.not_equal)
            qm = sb.tile([64, 64], F32, tag="qm")
            nc.vector.tensor_tensor(out=qm[:], in0=q_ps[:], in1=m2[:], op=ALU.mult)
            Dm2 = sb.tile([64, 64], F32, tag="Dm2")
            nc.vector.tensor_tensor(out=Dm2[:], in0=z2[:], in1=qm[:],
                                    op=ALU.subtract)

            # ---- backward: g1m = (w2b^T @ Dm2) * M1W, per chunk
            g1m = sb.tile([128, 4, 64], F32, tag="g1m")
            for t in range(4):
                g1_ps = ps.tile([128, 64], F32, tag="g128", bufs=4,
                                name=f"g1{t}")
                nc.tensor.matmul(g1_ps[:], w2b[:, t * 128:(t + 1) * 128],
                                 Dm2[:], start=True, stop=True)
                nc.vector.tensor_tensor(out=g1m[:, t, :], in0=g1_ps[:],
                                        in1=M1W[:, t, :], op=ALU.mult)

            # ---- C [100, 64] = sum_t Scomb_t^T @ g1m_t
            C_ps = ps.tile([100, 64], F32, tag="a1", bufs=1)
            for t in range(4):
                nc.tensor.matmul(C_ps[:], Scomb[:, t, :], g1m[:, t, :],
                                 start=(t == 0), stop=(t == 3))
            nc.vector.tensor_copy(out=C_ext[0:100, :], in_=C_ps[:])
            C_sb = C_ext[0:100, :]
            if debug:
                nc.sync.dma_start(out=probe("C", [100, 64])[:], in_=C_sb)

            # ---- e_min dance
            cand = [None] * 3
            for cc in range(3):
                cand[cc] = ps.tile([128, 8, 8], F32, tag="g128", bufs=4,
                                   name=f"cand{cc}")
                nc.tensor.matmul(
                    cand[cc][:].rearrange("k p q -> k (p q)"),
                    CandM[:, cc, :], C_ext[:], start=True, stop=True)
            # shift-aligned candidate stack (zero prefill = min-with-0
            # candidate); one innermost-axis min-reduce collapses 8 classes.
            eB = sb.tile([16, 12, 8], F32, tag="eB")
            nc.vector.memset(eB[:], 0.0)
            cstk = sb.tile([16, 8, 8, 10], F32, tag="cstk")
            nc.gpsimd.memset(cstk[:], 0.0)
            for j, k in enumerate([0, 1, 2, 3, 5, 6, 7, 8]):
                cc, kk = divmod(k, 4)
                dp, dq = k // 3 - 1, k % 3 - 1
                i4lo, i4hi = max(0, dp), min(8, 8 + dp)
                j4lo, j4hi = max(0, dq), min(8, 8 + dq)
                srcap = cand[cc][kk * 32:kk * 32 + 16,
                                 i4lo - dp:i4hi - dp,
                                 j4lo - dq:j4hi - dq, None]
                dstap = cstk[:, i4lo:i4hi, j4lo:j4hi, j:j + 1]
                if k % 2 == 0:
                    nc.scalar.copy(out=dstap, in_=srcap)
                else:
                    nc.vector.tensor_copy(out=dstap, in_=srcap)
            nc.vector.tensor_copy(out=cstk[:, :, :, 8:9],
                                  in_=cand[1][0:16, :, :, None])
            nc.vector.tensor_reduce(out=eB[:, 2:10, :], in_=cstk[:],
                                    axis=mybir.AxisListType.X, op=ALU.min)
            if debug:
                nc.sync.dma_start(out=probe("eB", [16, 96])[:],
                                  in_=eB[:].rearrange("a b c -> a (b c)"))
            eBf = eB[:].rearrange("a b c -> a (b c)")
            eW_ps = ps.tile([100, 64], F32, tag="S", bufs=1)
            for k in range(9):
                dp, dq = k // 3 - 1, k % 3 - 1
                off = 16 + 8 * dp + dq
                nc.tensor.matmul(eW_ps[:], PermB[:, k * 100:(k + 1) * 100],
                                 eBf[:, off:off + 64],
                                 start=(k == 0), stop=(k == 8))
            maskw = sb.tile([100, 64], F32, tag="maskw")
            nc.vector.tensor_tensor(out=maskw[:], in0=C_sb, in1=eW_ps[:],
                                    op=ALU.is_le)
            if debug:
                eW_sb = sb.tile([100, 64], F32, tag="eW_sb")
                nc.vector.tensor_copy(out=eW_sb[:], in_=eW_ps[:])
                nc.sync.dma_start(out=probe("eW", [100, 64])[:], in_=eW_sb[:])
            if debug:
                nc.sync.dma_start(out=probe("maskw", [100, 64])[:], in_=maskw[:])

            # ---- masked forward: Xm = X * maskw (broadcast over h)
            Xm = sb.tile([100, 3, 64], F32, tag="Xm")
            mask_b = bass.AP(tensor=maskw[:].tensor, offset=maskw[:].offset,
                             ap=[[64, 100], [0, 3], [1, 64]])
            nc.vector.tensor_tensor(out=Xm[:], in0=X, in1=mask_b, op=ALU.mult)
            u1m = sb.tile([128, 4, 64], F32, tag="u1m")
            for t in range(4):
                u1_ps = ps.tile([128, 64], F32, tag="g128", bufs=4,
                                name=f"u1{t}")
                for h in range(3):
                    nc.tensor.matmul(u1_ps[:], W1big[:, h, t, :], Xm[:, h, :],
                                     start=(h == 0), stop=(h == 2))
                nc.vector.tensor_tensor(out=u1m[:, t, :], in0=u1_ps[:],
                                        in1=M1W[:, t, :], op=ALU.mult)
            zm_ps = ps.tile([64, 64], F32, tag="q64", bufs=2)
            for t in range(4):
                nc.tensor.matmul(zm_ps[:], w2fT[:, t, :], u1m[:, t, :],
                                 start=(t == 0), stop=(t == 3))
            z2m = sb.tile([64, 64], F32, tag="z2m")
            nc.vector.tensor_tensor(out=z2m[:], in0=zm_ps[:], in1=m2[:],
                                    op=ALU.mult)
            if debug:
                nc.sync.dma_start(out=probe("z2m", [64, 64])[:], in_=z2m[:])

            # ---- Hopfield #2 -> output
            q2_ps = _hopfield(nc, sb, ps, z2m[:], KT_bf[:], KV, ident, "2")
            out_sb = sb.tile([64, 64], F32, tag="out_sb")
            nc.vector.tensor_copy(out=out_sb[:], in_=q2_ps[:])
            nc.sync.dma_start(out=out_t[:], in_=out_sb[:])
    nc.compile()
    return nc


def _get_nc(debug=False):
    key = ("nc", debug)
    if key not in _CACHE:
        _CACHE[key] = _build_nc(debug)
    return _CACHE[key]


# ---------------------------------------------------------------- entry point
def kernel(x, w1, b1, w2, b2, K, Vw, _debug=False):
    x = np.asarray(x, np.float32)
    shared = _host_prep(np.asarray(w1, np.float32), np.asarray(b1, np.float32),
                        np.asarray(w2, np.float32), np.asarray(b2, np.float32),
                        np.asarray(K, np.float32), np.asarray(Vw, np.float32))
    w1f, b1c = shared.pop("_w1f"), shared.pop("_b1")
    bsz = x.shape[0]
    nc = _get_nc(_debug)
    smpls = [_sample_prep(x[b], w1f, b1c) for b in range(bsz)]
    in_maps = []
    for core in range(N_CORES):
        cvb, xb = smpls[core] if core < bsz else smpls[0]
        m = dict(shared)
        m["cv"], m["smpl"] = cvb, xb
        in_maps.append(m)
    res = run_bass_kernel_spmd(nc, in_maps, core_ids=list(range(N_CORES)))
    out = np.stack([res.results[b]["out"].reshape(64, 8, 8)
                    for b in range(bsz)]).astype(np.float32)
    if _debug:
        return out, res
    return out



# revision 8
# speedup vs baseline: 1.0743x; 1.0743x over previous
"""TRN2 Bass kernel for nn_Block1_43542378447225 (v2).

Pipeline (per sample, one NeuronCore; batch=2 -> cores 0/1 do real work):
  conv1 -> relu -> conv2 -> relu (fp32, exact relu masks)
  Hopfield #1 in [m,pq] orientation (bf16 scores, ones-column row sums)
  backward (w2b matmul, mask, Scomb matmul) -> C in bf16
  blocked e_min via permutation matmuls (bf16, monotone-rounding-consistent)
  masked patch forward (bf16) -> z2_masked -> Hopfield #2 -> output

Host precomputes im2col patches, weight layouts, K@Vw (+ones column), all
packed into 4 DMA blobs (fp32 where relu-mask exactness matters, bf16
elsewhere); the device does all matmuls/softmax/masking.

Layout conventions:
  pq = p*8+q (64 output positions), uv = u*10+v (100 composite-window offsets)
  chunk t = conv2 kernel row kr, a = conv2 kernel col ks
  kc = t*128 + a*32 + c1 (hidden index, 4 chunks of 128 partitions)
  Hopfield memory chunks: m-chunk t = rows t*128:(t+1)*128 of the 512 codebook
"""
import numpy as np

import concourse.bass as bass
import concourse.bacc as bacc
import concourse.mybir as mybir
import concourse.tile as tile
from concourse.tile import add_dep_helper
from concourse.bass_utils import run_bass_kernel_spmd

F32 = mybir.dt.float32
BF16 = mybir.dt.bfloat16
AF = mybir.ActivationFunctionType
ALU = mybir.AluOpType

N_CORES = 8
BETA = 0.125  # 1/sqrt(64)

_CACHE = {}


# ---------------------------------------------------------------- host prep
def _bf16(a):
    """Round-to-nearest-even fp32 -> bf16 (ml_dtypes array for PJRT binding)."""
    import ml_dtypes
    return np.ascontiguousarray(a, np.float32).astype(ml_dtypes.bfloat16)


def _build_scomb_w1big(w1):
    w1s = w1.sum(axis=1)
    Scomb = np.zeros((4, 32, 4, 100), np.float32)  # [a, c1, t, uv]
    W1big = np.zeros((100, 3, 4, 4, 32), np.float32)  # [uv, h, t, a, c1]
    for t in range(4):
        for a in range(4):
            for u in range(10):
                ki = u - 2 * t
                if not (0 <= ki < 4):
                    continue
                for v in range(10):
                    kj = v - 2 * a
                    if not (0 <= kj < 4):
                        continue
                    Scomb[a, :, t, u * 10 + v] = w1s[:, ki, kj]
                    W1big[u * 10 + v, :, t, a, :] = w1[:, :, ki, kj].T
    # partition index = a*32+c1 -> merge (a, c1); free = t*100+uv
    Scomb = Scomb.reshape(128, 400)
    W1big = W1big.reshape(100, 1536)
    return Scomb, W1big


def _host_prep(w1, b1, w2, b2, K, Vw):
    Scomb, W1big = _build_scomb_w1big(w1)
    PermF = np.zeros((100, 9, 16), np.float32)
    for k in range(9):
        dp, dq = k // 3 - 1, k % 3 - 1
        for im in range(4):
            u = 4 * dp + im + 3
            if not (0 <= u < 10):
                continue
            for jm in range(4):
                v = 4 * dq + jm + 3
                if not (0 <= v < 10):
                    continue
                PermF[u * 10 + v, k, im * 4 + jm] = 1.0
    CandM = np.zeros((100, 3, 128), np.float32)
    for k in range(9):
        cc, kk = divmod(k, 4)
        CandM[:, cc, kk * 32:kk * 32 + 16] = PermF[:, k, :]
    PermB = np.transpose(PermF, (2, 1, 0)).reshape(16, 900)

    w2fT = np.transpose(w2, (3, 1, 2, 0)).reshape(128, 256)
    w2b = 2.0 * np.transpose(w2, (0, 2, 3, 1)).reshape(64, 512)
    KT = np.ascontiguousarray(K.T)                       # [64, 512]
    KVc = (K @ Vw).reshape(4, 128, 64)                   # m-chunks

    # wB64 [64, 1024] bf16: KT | w2b (both contract over c=64, base part 0)
    wB64 = np.zeros((64, 1024), np.float32)
    wB64[:, 0:512] = KT
    wB64[:, 512:1024] = w2b
    # wBF [128, 916] bf16: KV1 (4x65) | Scomb | w2fT_bf
    wBF = np.zeros((128, 916), np.float32)
    for t in range(4):
        wBF[:, t * 65:t * 65 + 64] = KVc[t]
        wBF[:, t * 65 + 64] = 1.0
    wBF[:, 260:660] = Scomb
    wBF[:, 660:916] = w2fT

    # wCW [100, 1920] bf16: W1big | CandM
    wCW = np.zeros((100, 1920), np.float32)
    wCW[:, 0:1536] = W1big
    wCW[:, 1536:1920] = CandM.reshape(100, 384)

    return {"wB64": _bf16(wB64), "wBF": _bf16(wBF),
            "wCW": _bf16(wCW), "wPB": _bf16(PermB),
            "_w2fT": w2fT, "_b2": b2,
            "_w1f": np.ascontiguousarray(np.transpose(w1, (2, 3, 1, 0)).reshape(48, 32)),
            "_b1": np.ascontiguousarray(b1[:, None])}


def _sample_prep(x_s, w1f, b1c, w2fT, b2):
    xp1 = np.pad(x_s, ((0, 0), (1, 1), (1, 1)))
    xp3 = np.pad(x_s, ((0, 0), (3, 3), (3, 3)))
    P1 = np.zeros((4, 4, 3, 16, 16), np.float32)
    for kr in range(4):
        for ks in range(4):
            P1[kr, ks] = xp1[:, kr:kr + 32:2, ks:ks + 32:2][:, :16, :16]
    X = np.zeros((10, 10, 3, 8, 8), np.float32)
    for u in range(10):
        for v in range(10):
            X[u, v] = xp3[:, u:u + 32:4, v:v + 32:4][:, :8, :8]
    # cvF [128, 738] fp32: P1 | w1f | b1 | w2fT | b2 | X
    cvF = np.zeros((128, 738), np.float32)
    cvF[0:48, 0:256] = P1.reshape(48, 256)
    cvF[0:48, 256:288] = w1f
    cvF[0:32, 288:289] = b1c
    cvF[:, 289:545] = w2fT
    cvF[0:64, 545:546] = b2[:, None]
    cvF[0:100, 546:738] = X.reshape(100, 192)
    return cvF


# ---------------------------------------------------------------- device build
def _hopfield2(nc, sb, ps, z_bf, KTp, KV1, tag):
    """z_bf [64(c), 64(pq)] bf16 -> (q_ps [65, 64] PSUM fp32).
    Scores computed directly in [m, pq] orientation (no transposes); the
    ones column folded into KV1 yields per-pq row sums in row 64."""
    S_ps = ps.tile([128, 4, 64], F32, tag="S", bufs=1, name=f"S{tag}")
    for t in range(4):
        nc.tensor.matmul(S_ps[:, t, :], KTp[:, t * 128:(t + 1) * 128], z_bf,
                         start=True, stop=True)
    E = sb.tile([128, 4, 64], BF16, tag=f"E{tag}", name=f"E{tag}")
    nc.scalar.activation(out=E[:], in_=S_ps[:], func=AF.Exp,
                         bias=0.0, scale=BETA)
    q_ps = ps.tile([65, 64], F32, tag="q65", bufs=1, name=f"q{tag}")
    for t in range(4):
        nc.tensor.matmul(q_ps[:], KV1[:, t, :], E[:, t, :],
                         start=(t == 0), stop=(t == 3))
    return q_ps


def _build_nc(debug=False):
    nc = bacc.Bacc("TRN2", target_bir_lowering=False, debug=False,
                   num_devices=N_CORES)
    d_cvF = nc.dram_tensor("cvF", [128, 738], F32, kind="ExternalInput")
    d_wB64 = nc.dram_tensor("wB64", [64, 1024], BF16, kind="ExternalInput")
    d_wBF = nc.dram_tensor("wBF", [128, 916], BF16, kind="ExternalInput")
    d_wCW = nc.dram_tensor("wCW", [100, 1920], BF16, kind="ExternalInput")
    d_wPB = nc.dram_tensor("wPB", [16, 900], BF16, kind="ExternalInput")
    out_t = nc.dram_tensor("out", [64, 64], F32, kind="ExternalOutput")

    with tile.TileContext(nc) as tc:
        with tc.tile_pool(name="sb", bufs=1) as sb, \
             tc.tile_pool(name="ps", bufs=1, space="PSUM") as ps:
            # ---- PE warm-up out of the cold p-state before conv1 arrives.
            warm = sb.tile([2, 8], F32, tag="warm")
            nc.vector.memset(warm[:], 0.0)
            for w_ in range(3):
                warm_ps = ps.tile([8, 8], F32, tag="qB", bufs=1,
                                  name=f"warm{w_}")
                nc.tensor.matmul(warm_ps[:], warm[0:2, :], warm[0:2, :],
                                 start=True, stop=True)

            # ---- loads: 2 HWDGE queues (SP: cvF+wCW, ACT: wBF+wPB)
            cvF = sb.tile([128, 738], F32, tag="cvF")
            nc.sync.dma_start(out=cvF[:], in_=d_cvF[:])
            wB64 = sb.tile([64, 1024], BF16, tag="wB64")
            nc.scalar.dma_start(out=wB64[:], in_=d_wB64[:])
            wBF = sb.tile([128, 916], BF16, tag="wBF")
            nc.scalar.dma_start(out=wBF[:], in_=d_wBF[:])
            wCW = sb.tile([100, 1920], BF16, tag="wCW")
            nc.sync.dma_start(out=wCW[:], in_=d_wCW[:])
            wPB = sb.tile([16, 900], BF16, tag="wPB")
            nc.scalar.dma_start(out=wPB[:], in_=d_wPB[:])

            P1 = cvF[0:48, 0:256]
            w1f = cvF[0:48, 256:288]
            b1 = cvF[0:32, 288:289]
            w2fT = cvF[:, 289:545].rearrange("k (t c) -> k t c", t=4)
            b2 = cvF[0:64, 545:546]
            X = cvF[0:100, 546:738].rearrange("u (h q) -> u h q", h=3)
            KTp = wB64[0:64, 0:512]
            w2bp = wB64[0:64, 512:1024]
            KV1 = wBF[:, 0:260].rearrange("k (t u) -> k t u", t=4)
            Scomb = wBF[:, 260:660].rearrange("k (t u) -> k t u", t=4)
            w2fT_bf = wBF[:, 660:916].rearrange("k (t c) -> k t c", t=4)
            W1big = wCW[0:100, 0:1536].rearrange("u (h t k) -> u h t k",
                                                 h=3, t=4)
            CandM = wCW[0:100, 1536:1920].rearrange("u (c k) -> u c k", c=3)
            PermB = wPB[0:16, :]

            # early memsets fill the initial DMA-wait window
            a1p = sb.tile([32, 18, 18], F32, tag="a1p")
            nc.vector.memset(a1p[:], 0.0)
            cstk = sb.tile([16, 8, 8, 10], BF16, tag="cstk")
            nc.gpsimd.memset(cstk[:], 0.0)
            eB = sb.tile([16, 12, 8], BF16, tag="eB")
            nc.vector.memset(eB[:], 0.0)
            ones_sb = sb.tile([1, 64], F32, tag="ones")
            nc.vector.memset(ones_sb[:], 1.0)

            # ---- conv1 + relu into padded a1p [32, 18, 18]
            a1_ps = ps.tile([32, 256], F32, tag="misc", bufs=2)
            nc.tensor.matmul(a1_ps[:], w1f, P1, start=True, stop=True)
            nc.scalar.activation(
                out=a1p[:, 1:17, 1:17],
                in_=a1_ps[:].rearrange("c (p q) -> c p q", p=16),
                func=AF.Relu, bias=b1, scale=1.0)

            # ---- P2 im2col: P2[a*32+c1, t, p, q] = a1p[c1, 2p+t, 2q+a]
            P2 = sb.tile([128, 4, 8, 8], F32, tag="P2")
            a1p_ap = a1p[:]
            for a in range(4):
                src = bass.AP(
                    tensor=a1p_ap.tensor,
                    offset=a1p_ap.offset + a,
                    ap=[[324, 32], [18, 4], [36, 8], [2, 8]])
                eng = (nc.vector, nc.scalar, nc.vector, nc.gpsimd)[a]
                if eng is nc.scalar:
                    eng.copy(out=P2[a * 32:(a + 1) * 32, :, :, :], in_=src)
                else:
                    eng.tensor_copy(out=P2[a * 32:(a + 1) * 32, :, :, :],
                                    in_=src)
            P2f = P2[:].rearrange("k t p q -> k t (p q)")
            M1W = sb.tile([128, 4, 64], F32, tag="M1W")
            nc.vector.tensor_scalar(out=M1W[:], in0=P2f, scalar1=0.0,
                                    scalar2=None, op0=ALU.not_equal)

            # ---- conv2 + relu -> z2 [64, 64] fp32 (exact relu mask), m2
            z2_ps = ps.tile([64, 64], F32, tag="misc", bufs=2)
            for t in range(4):
                nc.tensor.matmul(z2_ps[:], w2fT[:, t, :], P2f[:, t, :],
                                 start=(t == 0), stop=(t == 3))
            z2 = sb.tile([64, 64], F32, tag="z2")
            nc.scalar.activation(out=z2[:], in_=z2_ps[:],
                                 func=AF.Relu, bias=b2, scale=1.0)
            z_bf = sb.tile([64, 64], BF16, tag="z_bf")
            nc.scalar.copy(out=z_bf[:], in_=z2[:])
            m2 = sb.tile([64, 64], F32, tag="m2")
            nc.vector.tensor_scalar(out=m2[:], in0=z2[:], scalar1=0.0,
                                    scalar2=None, op0=ALU.not_equal)

            # ---- Hopfield #1 -> Dm2 = (z2 - q/s) * m2
            q1_ps = _hopfield2(nc, sb, ps, z_bf[:], KTp, KV1, "1")
            rec1 = sb.tile([1, 64], F32, tag="rec1")
            nc.vector.reciprocal(rec1[:], q1_ps[64:65, :])
            q1_sb = sb.tile([64, 64], F32, tag="q1_sb")
            nc.scalar.copy(out=q1_sb[:], in_=q1_ps[0:64, :])
            recB1_ps = ps.tile([64, 64], F32, tag="qB", bufs=1, name="recB1")
            nc.tensor.matmul(recB1_ps[:], ones_sb[:], rec1[:],
                             start=True, stop=True)
            qn = sb.tile([64, 64], F32, tag="qn")
            nc.vector.tensor_tensor(out=qn[:], in0=q1_sb[:],
                                    in1=recB1_ps[:], op=ALU.mult)
            dz = sb.tile([64, 64], F32, tag="dz")
            nc.vector.tensor_tensor(out=dz[:], in0=z2[:], in1=qn[:],
                                    op=ALU.subtract)
            Dm2 = sb.tile([64, 64], BF16, tag="Dm2")
            nc.vector.tensor_tensor(out=Dm2[:], in0=dz[:], in1=m2[:],
                                    op=ALU.mult)

            # ---- backward: g1m = (w2b^T @ Dm2) * M1W, per chunk (bf16)
            g1m = sb.tile([128, 4, 64], BF16, tag="g1m")
            for t in range(4):
                g1_ps = ps.tile([128, 64], F32, tag="g128", bufs=3,
                                name=f"g1{t}")
                nc.tensor.matmul(g1_ps[:], w2bp[:, t * 128:(t + 1) * 128],
                                 Dm2[:], start=True, stop=True)
                nc.vector.tensor_tensor(out=g1m[:, t, :], in0=g1_ps[:],
                                        in1=M1W[:, t, :], op=ALU.mult)

            # ---- C [100, 64] = sum_t Scomb_t^T @ g1m_t, in bf16
            C_ps = ps.tile([100, 64], F32, tag="misc", bufs=2)
            for t in range(4):
                nc.tensor.matmul(C_ps[:], Scomb[:, t, :], g1m[:, t, :],
                                 start=(t == 0), stop=(t == 3))
            C_bf = sb.tile([100, 64], BF16, tag="C_bf")
            nc.vector.tensor_copy(out=C_bf[:], in_=C_ps[:])

            # ---- e_min dance (all values exact bf16 copies of C entries,
            # so min/compare are consistent under monotone rounding)
            cand = [None] * 3
            for cc in range(3):
                cand[cc] = ps.tile([128, 8, 8], F32, tag="g128", bufs=3,
                                   name=f"cand{cc}")
                nc.tensor.matmul(
                    cand[cc][:].rearrange("k p q -> k (p q)"),
                    CandM[:, cc, :], C_bf[:], start=True, stop=True)
            # shift-aligned candidate stack (zero prefill = min-with-0
            # candidate); one innermost-axis min-reduce collapses 8 classes.
            for j, k in enumerate([0, 1, 2, 3, 5, 6, 7, 8]):
                cc, kk = divmod(k, 4)
                dp, dq = k // 3 - 1, k % 3 - 1
                i4lo, i4hi = max(0, dp), min(8, 8 + dp)
                j4lo, j4hi = max(0, dq), min(8, 8 + dq)
                srcap = cand[cc][kk * 32:kk * 32 + 16,
                                 i4lo - dp:i4hi - dp,
                                 j4lo - dq:j4hi - dq, None]
                dstap = cstk[:, i4lo:i4hi, j4lo:j4hi, j:j + 1]
                if j % 2 == 0:
                    nc.scalar.copy(out=dstap, in_=srcap)
                else:
                    nc.vector.tensor_copy(out=dstap, in_=srcap)
            nc.vector.tensor_copy(out=cstk[:, :, :, 8:9],
                                  in_=cand[1][0:16, :, :, None])
            nc.vector.tensor_reduce(out=eB[:, 2:10, :], in_=cstk[:],
                                    axis=mybir.AxisListType.X, op=ALU.min)
            eBf = eB[:].rearrange("a b c -> a (b c)")
            eW_ps = ps.tile([100, 64], F32, tag="misc", bufs=2)
            for k in range(9):
                dp, dq = k // 3 - 1, k % 3 - 1
                off = 16 + 8 * dp + dq
                nc.tensor.matmul(eW_ps[:], PermB[:, k * 100:(k + 1) * 100],
                                 eBf[:, off:off + 64],
                                 start=(k == 0), stop=(k == 8))
            maskw = sb.tile([100, 64], F32, tag="maskw")
            nc.vector.tensor_tensor(out=maskw[:], in0=C_bf[:], in1=eW_ps[:],
                                    op=ALU.is_le)

            # ---- masked forward: Xm = X * maskw (broadcast over h), bf16
            Xm = sb.tile([100, 3, 64], BF16, tag="Xm")
            mask_b = bass.AP(tensor=maskw[:].tensor, offset=maskw[:].offset,
                             ap=[[64, 100], [0, 3], [1, 64]])
            nc.vector.tensor_tensor(out=Xm[:], in0=X, in1=mask_b, op=ALU.mult)
            u1m = sb.tile([128, 4, 64], BF16, tag="u1m")
            for t in range(4):
                u1_ps = ps.tile([128, 64], F32, tag="g128", bufs=3,
                                name=f"u1{t}")
                for h in range(3):
                    nc.tensor.matmul(u1_ps[:], W1big[:, h, t, :], Xm[:, h, :],
                                     start=(h == 0), stop=(h == 2))
                nc.vector.tensor_tensor(out=u1m[:, t, :], in0=u1_ps[:],
                                        in1=M1W[:, t, :], op=ALU.mult)
            zm_ps = ps.tile([64, 64], F32, tag="misc", bufs=2)
            for t in range(4):
                nc.tensor.matmul(zm_ps[:], w2fT_bf[:, t, :], u1m[:, t, :],
                                 start=(t == 0), stop=(t == 3))
            z2m = sb.tile([64, 64], BF16, tag="z2m")
            nc.vector.tensor_tensor(out=z2m[:], in0=zm_ps[:], in1=m2[:],
                                    op=ALU.mult)

            # ---- Hopfield #2 -> output [c, pq] (normalized via recB trick)
            q2_ps = _hopfield2(nc, sb, ps, z2m[:], KTp, KV1, "2")
            rec2 = sb.tile([1, 64], F32, tag="rec2")
            nc.vector.reciprocal(rec2[:], q2_ps[64:65, :])
            q2_sb = sb.tile([64, 64], F32, tag="q2_sb")
            nc.scalar.copy(out=q2_sb[:], in_=q2_ps[0:64, :])
            recB2_ps = ps.tile([64, 64], F32, tag="qB", bufs=1, name="recB2")
            nc.tensor.matmul(recB2_ps[:], ones_sb[:], rec2[:],
                             start=True, stop=True)
            out_sb = sb.tile([64, 64], F32, tag="out_sb")
            nc.vector.tensor_tensor(out=out_sb[:], in0=q2_sb[:],
                                    in1=recB2_ps[:], op=ALU.mult)
            nc.sync.dma_start(out=out_t[:], in_=out_sb[:])
    nc.compile()
    return nc


def _get_nc(debug=False):
    key = ("nc", debug)
    if key not in _CACHE:
        _CACHE[key] = _build_nc(debug)
    return _CACHE[key]


# ---------------------------------------------------------------- entry point
def kernel(x, w1, b1, w2, b2, K, Vw, _debug=False):
    x = np.asarray(x, np.float32)
    shared = _host_prep(np.asarray(w1, np.float32), np.asarray(b1, np.float32),
                        np.asarray(w2, np.float32), np.asarray(b2, np.float32),
                        np.asarray(K, np.float32), np.asarray(Vw, np.float32))
    w1f, b1c = shared.pop("_w1f"), shared.pop("_b1")
    w2fT, b2h = shared.pop("_w2fT"), shared.pop("_b2")
    bsz = x.shape[0]
    nc = _get_nc(_debug)
    smpls = [_sample_prep(x[b], w1f, b1c, w2fT, b2h) for b in range(bsz)]
    in_maps = []
    for core in range(N_CORES):
        m = dict(shared)
        m["cvF"] = smpls[core] if core < bsz else smpls[0]
        in_maps.append(m)
    res = run_bass_kernel_spmd(nc, in_maps, core_ids=list(range(N_CORES)))
    out = np.stack([res.results[b]["out"].reshape(64, 8, 8)
                    for b in range(bsz)]).astype(np.float32)
    if _debug:
        return out, res
    return out


# revision 10
# speedup vs baseline: 1.1568x; 1.0768x over previous
"""TRN2 Bass kernel for nn_Block1_43542378447225 (v3).

Pipeline (per sample, one NeuronCore; batch=2 -> cores 0/1 do real work):
  conv1 -> relu -> conv2 -> relu (fp32, exact relu masks)
  Hopfield #1 in [m,pq] orientation (bf16 scores, ones-column row sums)
  backward (w2b matmul, mask, Scomb matmul) -> C in bf16
  blocked e_min via permutation matmuls (bf16, monotone-rounding-consistent)
  masked patch forward (bf16) -> z2_masked -> Hopfield #2 -> output

Host precomputes im2col patches, weight layouts, K@Vw (+ones column), packed
into 5 DMA blobs (fp32 where relu-mask exactness matters, bf16 elsewhere);
the device does all matmuls/exp/masking. The final softmax normalization
division (row 64 of the output carries the per-pq denominators) runs on host.

Layout conventions:
  pq = p*8+q (64 output positions), uv = u*10+v (100 composite-window offsets)
  chunk t = conv2 kernel row kr, a = conv2 kernel col ks
  kc = t*128 + a*32 + c1 (hidden index, 4 chunks of 128 partitions)
  Hopfield memory chunks: m-chunk t = rows t*128:(t+1)*128 of the 512 codebook
"""
import numpy as np

import concourse.bass as bass
import concourse.bacc as bacc
import concourse.mybir as mybir
import concourse.tile as tile
from concourse.bass_utils import run_bass_kernel_spmd

F32 = mybir.dt.float32
BF16 = mybir.dt.bfloat16
AF = mybir.ActivationFunctionType
ALU = mybir.AluOpType

N_CORES = 8
BETA = 0.125  # 1/sqrt(64)

_CACHE = {}


# ---------------------------------------------------------------- host prep
def _bf16(a):
    """Round-to-nearest-even fp32 -> bf16 (ml_dtypes array for PJRT binding)."""
    import ml_dtypes
    return np.ascontiguousarray(a, np.float32).astype(ml_dtypes.bfloat16)


def _build_scomb_w1big(w1):
    w1s = w1.sum(axis=1)
    Scomb = np.zeros((4, 32, 4, 100), np.float32)  # [a, c1, t, uv]
    W1big = np.zeros((100, 3, 4, 4, 32), np.float32)  # [uv, h, t, a, c1]
    for t in range(4):
        for a in range(4):
            for u in range(10):
                ki = u - 2 * t
                if not (0 <= ki < 4):
                    continue
                for v in range(10):
                    kj = v - 2 * a
                    if not (0 <= kj < 4):
                        continue
                    Scomb[a, :, t, u * 10 + v] = w1s[:, ki, kj]
                    W1big[u * 10 + v, :, t, a, :] = w1[:, :, ki, kj].T
    # partition index = a*32+c1 -> merge (a, c1); free = t*100+uv
    Scomb = Scomb.reshape(128, 400)
    W1big = W1big.reshape(100, 1536)
    return Scomb, W1big


def _host_prep(w1, b1, w2, b2, K, Vw):
    Scomb, W1big = _build_scomb_w1big(w1)
    PermF = np.zeros((100, 9, 16), np.float32)
    for k in range(9):
        dp, dq = k // 3 - 1, k % 3 - 1
        for im in range(4):
            u = 4 * dp + im + 3
            if not (0 <= u < 10):
                continue
            for jm in range(4):
                v = 4 * dq + jm + 3
                if not (0 <= v < 10):
                    continue
                PermF[u * 10 + v, k, im * 4 + jm] = 1.0
    CandM = np.zeros((100, 3, 128), np.float32)
    for k in range(9):
        cc, kk = divmod(k, 4)
        CandM[:, cc, kk * 32:kk * 32 + 16] = PermF[:, k, :]
    PermB = np.transpose(PermF, (2, 1, 0)).reshape(16, 900)

    w2fT = np.transpose(w2, (3, 1, 2, 0)).reshape(128, 256)
    w2b = 2.0 * np.transpose(w2, (0, 2, 3, 1)).reshape(64, 512)
    KT = np.ascontiguousarray(K.T)                       # [64, 512]
    KVc = (K @ Vw).reshape(4, 128, 64)                   # m-chunks

    # wB64 [64, 1024] bf16: KT | w2b (both contract over c=64, base part 0)
    wB64 = np.zeros((64, 1024), np.float32)
    wB64[:, 0:512] = KT
    wB64[:, 512:1024] = w2b
    # wBF [128, 916] bf16: KV1 (4x65) | Scomb | w2fT_bf
    wBF = np.zeros((128, 916), np.float32)
    for t in range(4):
        wBF[:, t * 65:t * 65 + 64] = KVc[t]
        wBF[:, t * 65 + 64] = 1.0
    wBF[:, 260:660] = Scomb
    wBF[:, 660:916] = w2fT

    # wCW [100, 1920] bf16: W1big | CandM
    wCW = np.zeros((100, 1920), np.float32)
    wCW[:, 0:1536] = W1big
    wCW[:, 1536:1920] = CandM.reshape(100, 384)

    return {"wB64": _bf16(wB64), "wBF": _bf16(wBF),
            "wCW": _bf16(wCW), "wPB": _bf16(PermB),
            "_w2fT": w2fT, "_b2": b2,
            "_w1f": np.ascontiguousarray(np.transpose(w1, (2, 3, 1, 0)).reshape(48, 32)),
            "_b1": np.ascontiguousarray(b1[:, None])}


def _sample_prep(x_s, w1f, b1c, w2fT, b2):
    xp1 = np.pad(x_s, ((0, 0), (1, 1), (1, 1)))
    xp3 = np.pad(x_s, ((0, 0), (3, 3), (3, 3)))
    P1 = np.zeros((4, 4, 3, 16, 16), np.float32)
    for kr in range(4):
        for ks in range(4):
            P1[kr, ks] = xp1[:, kr:kr + 32:2, ks:ks + 32:2][:, :16, :16]
    X = np.zeros((10, 10, 3, 8, 8), np.float32)
    for u in range(10):
        for v in range(10):
            X[u, v] = xp3[:, u:u + 32:4, v:v + 32:4][:, :8, :8]
    # cv1 [48, 289] fp32: P1 | w1f | b1  (first, smallest -> earliest conv1)
    cv1 = np.zeros((48, 289), np.float32)
    cv1[:, 0:256] = P1.reshape(48, 256)
    cv1[:, 256:288] = w1f
    cv1[0:32, 288:289] = b1c
    # cv2 [128, 449] fp32: w2fT | b2 | X
    cv2 = np.zeros((128, 449), np.float32)
    cv2[:, 0:256] = w2fT
    cv2[0:64, 256:257] = b2[:, None]
    cv2[0:100, 257:449] = X.reshape(100, 192)
    return cv1, cv2


# ---------------------------------------------------------------- device build
def _hopfield2(nc, sb, ps, z_bf, KTp, KV1, tag):
    """z_bf [64(c), 64(pq)] bf16 -> q_ps [65, 64] PSUM fp32.
    Scores computed directly in [m, pq] orientation (no transposes); the
    ones column folded into KV1 puts the per-pq exp-sum in row 64."""
    S_ps = ps.tile([128, 4, 64], F32, tag="S", bufs=1, name=f"S{tag}")
    for t in range(4):
        nc.tensor.matmul(S_ps[:, t, :], KTp[:, t * 128:(t + 1) * 128], z_bf,
                         start=True, stop=True)
    E = sb.tile([128, 4, 64], BF16, tag=f"E{tag}", name=f"E{tag}")
    nc.scalar.activation(out=E[:], in_=S_ps[:], func=AF.Exp,
                         bias=0.0, scale=BETA)
    q_ps = ps.tile([65, 64], F32, tag="q65", bufs=1, name=f"q{tag}")
    for t in range(4):
        nc.tensor.matmul(q_ps[:], KV1[:, t, :], E[:, t, :],
                         start=(t == 0), stop=(t == 3))
    return q_ps


def _build_nc(debug=False):
    nc = bacc.Bacc("TRN2", target_bir_lowering=False, debug=False,
                   num_devices=N_CORES)
    d_cv1 = nc.dram_tensor("cv1", [48, 289], F32, kind="ExternalInput")
    d_cv2 = nc.dram_tensor("cv2", [128, 449], F32, kind="ExternalInput")
    d_wB64 = nc.dram_tensor("wB64", [64, 1024], BF16, kind="ExternalInput")
    d_wBF = nc.dram_tensor("wBF", [128, 916], BF16, kind="ExternalInput")
    d_wCW = nc.dram_tensor("wCW", [100, 1920], BF16, kind="ExternalInput")
    d_wPB = nc.dram_tensor("wPB", [16, 900], BF16, kind="ExternalInput")
    out_t = nc.dram_tensor("out", [65, 64], F32, kind="ExternalOutput")

    with tile.TileContext(nc) as tc:
        with tc.tile_pool(name="sb", bufs=1) as sb, \
             tc.tile_pool(name="ps", bufs=1, space="PSUM") as ps:
            # ---- PE warm-up out of the cold p-state before conv1 arrives.
            # warm tile memset on Pool so the first matmul issues early.
            warm = sb.tile([2, 8], F32, tag="warm")
            nc.gpsimd.memset(warm[:], 0.0)
            for w_ in range(3):
                warm_ps = ps.tile([8, 8], F32, tag="qB", bufs=1,
                                  name=f"warm{w_}")
                nc.tensor.matmul(warm_ps[:], warm[0:2, :], warm[0:2, :],
                                 start=True, stop=True)

            # ---- loads (HWDGE order == need order)
            cv1 = sb.tile([48, 289], F32, tag="cv1")
            nc.sync.dma_start(out=cv1[:], in_=d_cv1[:])
            wB64 = sb.tile([64, 1024], BF16, tag="wB64")
            nc.scalar.dma_start(out=wB64[:], in_=d_wB64[:])
            cv2 = sb.tile([128, 449], F32, tag="cv2")
            nc.sync.dma_start(out=cv2[:], in_=d_cv2[:])
            wBF = sb.tile([128, 916], BF16, tag="wBF")
            nc.scalar.dma_start(out=wBF[:], in_=d_wBF[:])
            wCW = sb.tile([100, 1920], BF16, tag="wCW")
            nc.sync.dma_start(out=wCW[:], in_=d_wCW[:])
            wPB = sb.tile([16, 900], BF16, tag="wPB")
            nc.scalar.dma_start(out=wPB[:], in_=d_wPB[:])

            P1 = cv1[0:48, 0:256]
            w1f = cv1[0:48, 256:288]
            b1 = cv1[0:32, 288:289]
            w2fT = cv2[:, 0:256].rearrange("k (t c) -> k t c", t=4)
            b2 = cv2[0:64, 256:257]
            X = cv2[0:100, 257:449].rearrange("u (h q) -> u h q", h=3)
            KTp = wB64[0:64, 0:512]
            w2bp = wB64[0:64, 512:1024]
            KV1 = wBF[:, 0:260].rearrange("k (t u) -> k t u", t=4)
            Scomb = wBF[:, 260:660].rearrange("k (t u) -> k t u", t=4)
            w2fT_bf = wBF[:, 660:916].rearrange("k (t c) -> k t c", t=4)
            W1big = wCW[0:100, 0:1536].rearrange("u (h t k) -> u h t k",
                                                 h=3, t=4)
            CandM = wCW[0:100, 1536:1920].rearrange("u (c k) -> u c k", c=3)
            PermB = wPB[0:16, :]

            # early memsets fill the initial DMA-wait window
            cstk = sb.tile([16, 8, 8, 10], BF16, tag="cstk")
            nc.gpsimd.memset(cstk[:], 0.0)
            a1p = sb.tile([32, 18, 18], F32, tag="a1p")
            nc.gpsimd.memset(a1p[:], 0.0)
            eB = sb.tile([16, 12, 8], BF16, tag="eB")
            nc.vector.memset(eB[:], 0.0)
            ones_sb = sb.tile([1, 64], F32, tag="ones")
            nc.vector.memset(ones_sb[:], 1.0)

            # ---- conv1 + relu into padded a1p [32, 18, 18]
            a1_ps = ps.tile([32, 256], F32, tag="misc", bufs=2)
            nc.tensor.matmul(a1_ps[:], w1f, P1, start=True, stop=True)
            nc.scalar.activation(
                out=a1p[:, 1:17, 1:17],
                in_=a1_ps[:].rearrange("c (p q) -> c p q", p=16),
                func=AF.Relu, bias=b1, scale=1.0)

            # ---- P2 im2col: P2[a*32+c1, t, p, q] = a1p[c1, 2p+t, 2q+a]
            P2 = sb.tile([128, 4, 8, 8], F32, tag="P2")
            a1p_ap = a1p[:]
            for a in range(4):
                src = bass.AP(
                    tensor=a1p_ap.tensor,
                    offset=a1p_ap.offset + a,
                    ap=[[324, 32], [18, 4], [36, 8], [2, 8]])
                eng = (nc.vector, nc.scalar, nc.vector, nc.gpsimd)[a]
                if eng is nc.scalar:
                    eng.copy(out=P2[a * 32:(a + 1) * 32, :, :, :], in_=src)
                else:
                    eng.tensor_copy(out=P2[a * 32:(a + 1) * 32, :, :, :],
                                    in_=src)
            P2f = P2[:].rearrange("k t p q -> k t (p q)")
            M1W = sb.tile([128, 4, 64], F32, tag="M1W")
            nc.vector.tensor_scalar(out=M1W[:], in0=P2f, scalar1=0.0,
                                    scalar2=None, op0=ALU.not_equal)

            # ---- conv2 -> z2 (exact relu masks); z_bf on Act, z2 on DVE
            z2_ps = ps.tile([64, 64], F32, tag="misc", bufs=2)
            for t in range(4):
                nc.tensor.matmul(z2_ps[:], w2fT[:, t, :], P2f[:, t, :],
                                 start=(t == 0), stop=(t == 3))
            z_bf = sb.tile([64, 64], BF16, tag="z_bf")
            nc.scalar.activation(out=z_bf[:], in_=z2_ps[:],
                                 func=AF.Relu, bias=b2, scale=1.0)
            z2 = sb.tile([64, 64], F32, tag="z2")
            nc.vector.tensor_scalar(out=z2[:], in0=z2_ps[:], scalar1=b2,
                                    scalar2=0.0, op0=ALU.add, op1=ALU.max)
            m2 = sb.tile([64, 64], F32, tag="m2")
            nc.vector.tensor_scalar(out=m2[:], in0=z2[:], scalar1=0.0,
                                    scalar2=None, op0=ALU.not_equal)

            # ---- Hopfield #1 -> Dm2 = z2 - (q*m2)*recB   (diag commute)
            q1_ps = _hopfield2(nc, sb, ps, z_bf[:], KTp, KV1, "1")
            rec1 = sb.tile([1, 64], F32, tag="rec1")
            nc.vector.reciprocal(rec1[:], q1_ps[64:65, :])
            qm = sb.tile([64, 64], F32, tag="qm")
            nc.vector.tensor_tensor(out=qm[:], in0=q1_ps[0:64, :],
                                    in1=m2[:], op=ALU.mult)
            recB1_ps = ps.tile([64, 64], F32, tag="qB", bufs=1, name="recB1")
            nc.tensor.matmul(recB1_ps[:], ones_sb[:], rec1[:],
                             start=True, stop=True)
            qmB = sb.tile([64, 64], F32, tag="qmB")
            nc.vector.tensor_tensor(out=qmB[:], in0=qm[:],
                                    in1=recB1_ps[:], op=ALU.mult)
            Dm2 = sb.tile([64, 64], BF16, tag="Dm2")
            nc.vector.tensor_tensor(out=Dm2[:], in0=z2[:], in1=qmB[:],
                                    op=ALU.subtract)

            # ---- backward: g1m = (w2b^T @ Dm2) * M1W (fused mask mult)
            g1_ps = ps.tile([128, 4, 64], F32, tag="gf", bufs=2, name="g1")
            for t in range(4):
                nc.tensor.matmul(g1_ps[:, t, :],
                                 w2bp[:, t * 128:(t + 1) * 128],
                                 Dm2[:], start=True, stop=True)
            g1m = sb.tile([128, 4, 64], BF16, tag="g1m")
            nc.vector.tensor_tensor(out=g1m[:], in0=g1_ps[:], in1=M1W[:],
                                    op=ALU.mult)

            # ---- C [100, 64] = sum_t Scomb_t^T @ g1m_t, in bf16
            C_ps = ps.tile([100, 64], F32, tag="misc", bufs=2)
            for t in range(4):
                nc.tensor.matmul(C_ps[:], Scomb[:, t, :], g1m[:, t, :],
                                 start=(t == 0), stop=(t == 3))
            C_bf = sb.tile([100, 64], BF16, tag="C_bf")
            nc.vector.tensor_copy(out=C_bf[:], in_=C_ps[:])

            # ---- e_min dance (all values exact bf16 copies of C entries,
            # so min/compare are consistent under monotone rounding)
            cand_ps = ps.tile([128, 3, 8, 8], F32, tag="gf", bufs=2,
                              name="cand")
            for cc in range(3):
                nc.tensor.matmul(
                    cand_ps[:, cc, :, :].rearrange("k p q -> k (p q)"),
                    CandM[:, cc, :], C_bf[:], start=True, stop=True)
            cand_sb = sb.tile([128, 3, 8, 8], BF16, tag="cand_sb")
            nc.vector.tensor_copy(out=cand_sb[:], in_=cand_ps[:])
            # shift-aligned candidate stack (zero prefill = min-with-0
            # candidate); innermost-axis min-reduce collapses the 8 classes.
            for j, k in enumerate([0, 1, 2, 3, 5, 6, 7, 8]):
                cc, kk = divmod(k, 4)
                dp, dq = k // 3 - 1, k % 3 - 1
                i4lo, i4hi = max(0, dp), min(8, 8 + dp)
                j4lo, j4hi = max(0, dq), min(8, 8 + dq)
                srcap = cand_sb[kk * 32:kk * 32 + 16, cc,
                                i4lo - dp:i4hi - dp,
                                j4lo - dq:j4hi - dq, None]
                dstap = cstk[:, i4lo:i4hi, j4lo:j4hi, j:j + 1]
                eng = (nc.vector, nc.scalar, nc.gpsimd)[j % 3]
                if eng is nc.scalar:
                    eng.copy(out=dstap, in_=srcap)
                else:
                    eng.tensor_copy(out=dstap, in_=srcap)
            nc.scalar.copy(out=cstk[:, :, :, 8:9],
                           in_=cand_sb[0:16, 1, :, :, None])
            nc.vector.tensor_reduce(out=eB[:, 2:10, :], in_=cstk[:],
                                    axis=mybir.AxisListType.X, op=ALU.min)
            eBf = eB[:].rearrange("a b c -> a (b c)")
            eW_ps = ps.tile([100, 64], F32, tag="misc", bufs=2)
            for k in range(9):
                dp, dq = k // 3 - 1, k % 3 - 1
                off = 16 + 8 * dp + dq
                nc.tensor.matmul(eW_ps[:], PermB[:, k * 100:(k + 1) * 100],
                                 eBf[:, off:off + 64],
                                 start=(k == 0), stop=(k == 8))
            maskw = sb.tile([100, 64], F32, tag="maskw")
            nc.vector.tensor_tensor(out=maskw[:], in0=C_bf[:], in1=eW_ps[:],
                                    op=ALU.is_le)

            # ---- masked forward: Xm = X * maskw (broadcast over h), bf16
            Xm = sb.tile([100, 3, 64], BF16, tag="Xm")
            mask_b = bass.AP(tensor=maskw[:].tensor, offset=maskw[:].offset,
                             ap=[[64, 100], [0, 3], [1, 64]])
            nc.vector.tensor_tensor(out=Xm[:], in0=X, in1=mask_b, op=ALU.mult)
            u1_ps = ps.tile([128, 4, 64], F32, tag="gf", bufs=2, name="u1")
            for t in range(4):
                for h in range(3):
                    nc.tensor.matmul(u1_ps[:, t, :], W1big[:, h, t, :],
                                     Xm[:, h, :],
                                     start=(h == 0), stop=(h == 2))
            u1m = sb.tile([128, 4, 64], BF16, tag="u1m")
            nc.vector.tensor_tensor(out=u1m[:], in0=u1_ps[:], in1=M1W[:],
                                    op=ALU.mult)
            zm_ps = ps.tile([64, 64], F32, tag="misc", bufs=2)
            for t in range(4):
                nc.tensor.matmul(zm_ps[:], w2fT_bf[:, t, :], u1m[:, t, :],
                                 start=(t == 0), stop=(t == 3))
            z2m = sb.tile([64, 64], BF16, tag="z2m")
            nc.vector.tensor_tensor(out=z2m[:], in0=zm_ps[:], in1=m2[:],
                                    op=ALU.mult)

            # ---- Hopfield #2 -> output [65, 64]: rows 0:64 unnormalized
            # exp-weighted sums, row 64 the per-pq denominators (host divides)
            q2_ps = _hopfield2(nc, sb, ps, z2m[:], KTp, KV1, "2")
            out_sb = sb.tile([65, 64], F32, tag="out_sb")
            nc.vector.tensor_copy(out=out_sb[:], in_=q2_ps[:])
            nc.sync.dma_start(out=out_t[:], in_=out_sb[:])
    nc.compile()
    return nc


def _get_nc(debug=False):
    key = ("nc", debug)
    if key not in _CACHE:
        _CACHE[key] = _build_nc(debug)
    return _CACHE[key]


# ---------------------------------------------------------------- entry point
def kernel(x, w1, b1, w2, b2, K, Vw, _debug=False):
    x = np.asarray(x, np.float32)
    shared = _host_prep(np.asarray(w1, np.float32), np.asarray(b1, np.float32),
                        np.asarray(w2, np.float32), np.asarray(b2, np.float32),
                        np.asarray(K, np.float32), np.asarray(Vw, np.float32))
    w1f, b1c = shared.pop("_w1f"), shared.pop("_b1")
    w2fT, b2h = shared.pop("_w2fT"), shared.pop("_b2")
    bsz = x.shape[0]
    nc = _get_nc(_debug)
    smpls = [_sample_prep(x[b], w1f, b1c, w2fT, b2h) for b in range(bsz)]
    in_maps = []
    for core in range(N_CORES):
        m = dict(shared)
        m["cv1"], m["cv2"] = smpls[core] if core < bsz else smpls[0]
        in_maps.append(m)
    res = run_bass_kernel_spmd(nc, in_maps, core_ids=list(range(N_CORES)))
    outs = []
    for b in range(bsz):
        q2 = res.results[b]["out"].astype(np.float32)
        outs.append((q2[0:64] / q2[64:65]).reshape(64, 8, 8))
    out = np.stack(outs).astype(np.float32)
    if _debug:
        return out, res
    return out


# revision 14
# speedup vs baseline: 1.1874x; 1.0264x over previous
"""TRN2 Bass kernel for nn_Block1_43542378447225 (v4).

Pipeline (per sample, one NeuronCore; batch=2 -> cores 0/1 do real work):
  conv1 -> relu -> conv2 -> relu (bf16 matmuls, fp32 psum)
  Hopfield #1 in [m,pq] orientation (bf16 scores, ones-column row sums)
  backward split C = C_z - C_q*diag(1/s): the z2 half runs inside Hopfield
  #1's latency shadow; the q half needs no normalization until the very end
  blocked e_min via permutation matmuls (bf16, monotone-rounding-consistent)
  masked patch forward (bf16) -> z2_masked -> Hopfield #2 -> output

Host precomputes im2col patches, weight layouts, K@Vw (+ones column), packed
into 6 DMA blobs; the device does all matmuls/exp/masking. The final softmax
normalization division (row 64 of the output carries the per-pq denominators)
runs on host.

Layout conventions:
  pq = p*8+q (64 output positions), uv = u*10+v (100 composite-window offsets)
  chunk t = conv2 kernel row kr, a = conv2 kernel col ks
  kc = t*128 + a*32 + c1 (hidden index, 4 chunks of 128 partitions)
  Hopfield memory chunks: m-chunk t = rows t*128:(t+1)*128 of the 512 codebook
"""
import numpy as np

import concourse.bass as bass
import concourse.bacc as bacc
import concourse.mybir as mybir
import concourse.tile as tile
from concourse.bass_utils import run_bass_kernel_spmd

F32 = mybir.dt.float32
BF16 = mybir.dt.bfloat16
AF = mybir.ActivationFunctionType
ALU = mybir.AluOpType

N_CORES = 8
BETA = 0.125  # 1/sqrt(64)

_CACHE = {}


# ---------------------------------------------------------------- host prep
def _bf16(a):
    """Round-to-nearest-even fp32 -> bf16 (ml_dtypes array for PJRT binding)."""
    import ml_dtypes
    return np.ascontiguousarray(a, np.float32).astype(ml_dtypes.bfloat16)


def _build_scomb_w1big(w1):
    w1s = w1.sum(axis=1)
    Scomb = np.zeros((4, 32, 4, 100), np.float32)  # [a, c1, t, uv]
    W1big = np.zeros((100, 3, 4, 4, 32), np.float32)  # [uv, h, t, a, c1]
    for t in range(4):
        for a in range(4):
            for u in range(10):
                ki = u - 2 * t
                if not (0 <= ki < 4):
                    continue
                for v in range(10):
                    kj = v - 2 * a
                    if not (0 <= kj < 4):
                        continue
                    Scomb[a, :, t, u * 10 + v] = w1s[:, ki, kj]
                    W1big[u * 10 + v, :, t, a, :] = w1[:, :, ki, kj].T
    # partition index = a*32+c1 -> merge (a, c1); free = t*100+uv
    Scomb = Scomb.reshape(128, 400)
    W1big = W1big.reshape(100, 1536)
    return Scomb, W1big


def _host_prep(w1, b1, w2, b2, K, Vw):
    Scomb, W1big = _build_scomb_w1big(w1)
    PermF = np.zeros((100, 9, 16), np.float32)
    for k in range(9):
        dp, dq = k // 3 - 1, k % 3 - 1
        for im in range(4):
            u = 4 * dp + im + 3
            if not (0 <= u < 10):
                continue
            for jm in range(4):
                v = 4 * dq + jm + 3
                if not (0 <= v < 10):
                    continue
                PermF[u * 10 + v, k, im * 4 + jm] = 1.0
    CandM = np.zeros((100, 3, 128), np.float32)
    for k in range(9):
        cc, kk = divmod(k, 4)
        CandM[:, cc, kk * 32:kk * 32 + 16] = PermF[:, k, :]
    PermB = np.transpose(PermF, (2, 1, 0)).reshape(16, 900)

    w2fT = np.transpose(w2, (3, 1, 2, 0)).reshape(128, 256)
    w2b = 2.0 * np.transpose(w2, (0, 2, 3, 1)).reshape(64, 512)
    KT = np.ascontiguousarray(K.T)                       # [64, 512]
    KVc = (K @ Vw).reshape(4, 128, 64)                   # m-chunks

    # wB64 [64, 1024] bf16: KT | w2b (both contract over c=64, base part 0)
    wB64 = np.zeros((64, 1024), np.float32)
    wB64[:, 0:512] = KT
    wB64[:, 512:1024] = w2b
    # wBF [128, 916] bf16: KV1 (4x65) | Scomb | w2fT
    wBF = np.zeros((128, 916), np.float32)
    for t in range(4):
        wBF[:, t * 65:t * 65 + 64] = KVc[t]
        wBF[:, t * 65 + 64] = 1.0
    wBF[:, 260:660] = Scomb
    wBF[:, 660:916] = w2fT

    # wCW [100, 1920] bf16: W1big | CandM
    wCW = np.zeros((100, 1920), np.float32)
    wCW[:, 0:1536] = W1big
    wCW[:, 1536:1920] = CandM.reshape(100, 384)

    return {"wB64": _bf16(wB64), "wBF": _bf16(wBF),
            "wCW": _bf16(wCW), "wPB": _bf16(PermB),
            "_b2": b2,
            "_w1f": np.ascontiguousarray(np.transpose(w1, (2, 3, 1, 0)).reshape(48, 32)),
            "_b1": np.ascontiguousarray(b1[:, None])}


def _sample_prep(x_s, w1f, b1c, b2):
    xp1 = np.pad(x_s, ((0, 0), (1, 1), (1, 1)))
    xp3 = np.pad(x_s, ((0, 0), (3, 3), (3, 3)))
    P1 = np.zeros((4, 4, 3, 16, 16), np.float32)
    for kr in range(4):
        for ks in range(4):
            P1[kr, ks] = xp1[:, kr:kr + 32:2, ks:ks + 32:2][:, :16, :16]
    X = np.zeros((10, 10, 3, 8, 8), np.float32)
    for u in range(10):
        for v in range(10):
            X[u, v] = xp3[:, u:u + 32:4, v:v + 32:4][:, :8, :8]
    # cv1 [48, 288] bf16: P1 | w1f  (smallest, first -> earliest conv1)
    cv1 = np.zeros((48, 288), np.float32)
    cv1[:, 0:256] = P1.reshape(48, 256)
    cv1[:, 256:288] = w1f
    # cv2 [100, 194] fp32: b2 | b1 | X
    cv2 = np.zeros((100, 194), np.float32)
    cv2[0:64, 0:1] = b2[:, None]
    cv2[0:32, 1:2] = b1c
    cv2[:, 2:194] = X.reshape(100, 192)
    return _bf16(cv1), cv2


# ---------------------------------------------------------------- device build
def _hopfield2(nc, sb, ps, z_bf, KTp, KV1, tag):
    """z_bf [64(c), 64(pq)] bf16 -> q_ps [65, 64] PSUM fp32.
    Scores computed directly in [m, pq] orientation (no transposes); the
    ones column folded into KV1 puts the per-pq exp-sum in row 64."""
    S_ps = ps.tile([128, 4, 64], F32, tag="S", bufs=1, name=f"S{tag}")
    for t in range(4):
        nc.tensor.matmul(S_ps[:, t, :], KTp[:, t * 128:(t + 1) * 128], z_bf,
                         start=True, stop=True)
    E = sb.tile([128, 4, 64], BF16, tag=f"E{tag}", name=f"E{tag}")
    nc.scalar.activation(out=E[:], in_=S_ps[:], func=AF.Exp,
                         bias=0.0, scale=BETA)
    q_ps = ps.tile([65, 64], F32, tag="q65", bufs=1, name=f"q{tag}")
    for t in range(4):
        nc.tensor.matmul(q_ps[:], KV1[:, t, :], E[:, t, :],
                         start=(t == 0), stop=(t == 3))
    return q_ps


def _build_nc(debug=False):
    nc = bacc.Bacc("TRN2", target_bir_lowering=False, debug=False,
                   num_devices=N_CORES)
    d_cv1 = nc.dram_tensor("cv1", [48, 288], BF16, kind="ExternalInput")
    d_cv2 = nc.dram_tensor("cv2", [100, 194], F32, kind="ExternalInput")
    d_wB64 = nc.dram_tensor("wB64", [64, 1024], BF16, kind="ExternalInput")
    d_wBF = nc.dram_tensor("wBF", [128, 916], BF16, kind="ExternalInput")
    d_wCW = nc.dram_tensor("wCW", [100, 1920], BF16, kind="ExternalInput")
    d_wPB = nc.dram_tensor("wPB", [16, 900], BF16, kind="ExternalInput")
    out_t = nc.dram_tensor("out", [65, 64], F32, kind="ExternalOutput")

    with tile.TileContext(nc) as tc:
        with tc.tile_pool(name="sb", bufs=1) as sb, \
             tc.tile_pool(name="ps", bufs=1, space="PSUM") as ps:
            # ---- PE warm-up out of the cold p-state before conv1 arrives.
            warm = sb.tile([2, 8], F32, tag="warm")
            nc.gpsimd.memset(warm[:], 0.0)
            for w_ in range(3):
                warm_ps = ps.tile([8, 8], F32, tag="qB", bufs=1,
                                  name=f"warm{w_}")
                nc.tensor.matmul(warm_ps[:], warm[0:2, :], warm[0:2, :],
                                 start=True, stop=True)

            # ---- loads (SP queue: need order; Pool queue: PermB via SWDGE;
            # Act stays free of DMA configs so the first relu isn't delayed)
            cv1 = sb.tile([48, 288], BF16, tag="cv1")
            nc.sync.dma_start(out=cv1[:], in_=d_cv1[:])
            cv2 = sb.tile([100, 194], F32, tag="cv2")
            nc.sync.dma_start(out=cv2[:], in_=d_cv2[:])
            wB64 = sb.tile([64, 1024], BF16, tag="wB64")
            nc.sync.dma_start(out=wB64[:], in_=d_wB64[:])
            wBF = sb.tile([128, 916], BF16, tag="wBF")
            nc.sync.dma_start(out=wBF[:], in_=d_wBF[:])
            wCW = sb.tile([100, 1920], BF16, tag="wCW")
            nc.sync.dma_start(out=wCW[:], in_=d_wCW[:])
            wPB = sb.tile([16, 900], BF16, tag="wPB")
            nc.gpsimd.dma_start(out=wPB[:], in_=d_wPB[:])

            P1 = cv1[0:48, 0:256]
            w1f = cv1[0:48, 256:288]
            b2 = cv2[0:64, 0:1]
            b1 = cv2[0:32, 1:2]
            X = cv2[0:100, 2:194].rearrange("u (h q) -> u h q", h=3)
            KTp = wB64[0:64, 0:512]
            w2bp = wB64[0:64, 512:1024]
            KV1 = wBF[:, 0:260].rearrange("k (t u) -> k t u", t=4)
            Scomb = wBF[:, 260:660].rearrange("k (t u) -> k t u", t=4)
            w2fT = wBF[:, 660:916].rearrange("k (t c) -> k t c", t=4)
            W1big = wCW[0:100, 0:1536].rearrange("u (h t k) -> u h t k",
                                                 h=3, t=4)
            CandM = wCW[0:100, 1536:1920].rearrange("u (c k) -> u c k", c=3)
            PermB = wPB[0:16, :]

            # early memsets fill the initial DMA-wait window
            cstk = sb.tile([16, 8, 8, 10], BF16, tag="cstk")
            nc.gpsimd.memset(cstk[:], 0.0)
            a1p = sb.tile([32, 18, 18], BF16, tag="a1p")
            nc.gpsimd.memset(a1p[:], 0.0)
            eB = sb.tile([16, 12, 8], BF16, tag="eB")
            nc.vector.memset(eB[:], 0.0)
            ones_sb = sb.tile([1, 100], F32, tag="ones")
            nc.vector.memset(ones_sb[:], 1.0)

            # ---- conv1 + relu into padded a1p [32, 18, 18] (bf16)
            a1_ps = ps.tile([32, 256], F32, tag="misc", bufs=2)
            nc.tensor.matmul(a1_ps[:], w1f, P1, start=True, stop=True)
            nc.scalar.activation(
                out=a1p[:, 1:17, 1:17],
                in_=a1_ps[:].rearrange("c (p q) -> c p q", p=16),
                func=AF.Relu, bias=b1, scale=1.0)

            # ---- P2 im2col: P2[a*32+c1, t, p, q] = a1p[c1, 2p+t, 2q+a]
            P2 = sb.tile([128, 4, 8, 8], BF16, tag="P2")
            a1p_ap = a1p[:]
            for a in range(4):
                src = bass.AP(
                    tensor=a1p_ap.tensor,
                    offset=a1p_ap.offset + a,
                    ap=[[324, 32], [18, 4], [36, 8], [2, 8]])
                eng = (nc.vector, nc.scalar, nc.vector, nc.gpsimd)[a]
                if eng is nc.scalar:
                    eng.copy(out=P2[a * 32:(a + 1) * 32, :, :, :], in_=src)
                else:
                    eng.tensor_copy(out=P2[a * 32:(a + 1) * 32, :, :, :],
                                    in_=src)
            P2f = P2[:].rearrange("k t p q -> k t (p q)")
            M1W = sb.tile([128, 4, 64], F32, tag="M1W")
            nc.vector.tensor_scalar(out=M1W[:], in0=P2f, scalar1=0.0,
                                    scalar2=None, op0=ALU.not_equal)

            # ---- conv2 -> z2 (bf16 matmul); z_bf on Act, z2 fp32 on DVE
            z2_ps = ps.tile([64, 64], F32, tag="misc", bufs=2)
            for t in range(4):
                nc.tensor.matmul(z2_ps[:], w2fT[:, t, :], P2f[:, t, :],
                                 start=(t == 0), stop=(t == 3))
            z_bf = sb.tile([64, 64], BF16, tag="z_bf")
            nc.scalar.activation(out=z_bf[:], in_=z2_ps[:],
                                 func=AF.Relu, bias=b2, scale=1.0)
            z2 = sb.tile([64, 64], F32, tag="z2")
            nc.vector.tensor_scalar(out=z2[:], in0=z2_ps[:], scalar1=b2,
                                    scalar2=0.0, op0=ALU.add, op1=ALU.max)
            m2 = sb.tile([64, 64], F32, tag="m2")
            nc.vector.tensor_scalar(out=m2[:], in0=z2[:], scalar1=0.0,
                                    scalar2=None, op0=ALU.not_equal)

            # ---- Hopfield #1 (PE also squeezes the C_z half of the
            # backward into the latency shadow: C = C_z - C_q * diag(1/s))
            S1_ps = ps.tile([128, 4, 64], F32, tag="S", bufs=1, name="S1")
            for t in range(4):
                nc.tensor.matmul(S1_ps[:, t, :], KTp[:, t * 128:(t + 1) * 128],
                                 z_bf[:], start=True, stop=True)
            # z2 half of the backward (rhs z_bf, no Hopfield dependency)
            g1z_ps = ps.tile([128, 4, 64], F32, tag="gf", bufs=2, name="g1z")
            for t in range(4):
                nc.tensor.matmul(g1z_ps[:, t, :],
                                 w2bp[:, t * 128:(t + 1) * 128],
                                 z_bf[:], start=True, stop=True)
            E1 = sb.tile([128, 4, 64], BF16, tag="E1", name="E1")
            nc.scalar.activation(out=E1[:], in_=S1_ps[:], func=AF.Exp,
                                 bias=0.0, scale=BETA)
            g1zm = sb.tile([128, 4, 64], BF16, tag="g1zm")
            nc.vector.tensor_tensor(out=g1zm[:], in0=g1z_ps[:], in1=M1W[:],
                                    op=ALU.mult)
            Cz_ps = ps.tile([100, 64], F32, tag="misc", bufs=2)
            for t in range(4):
                nc.tensor.matmul(Cz_ps[:], Scomb[:, t, :], g1zm[:, t, :],
                                 start=(t == 0), stop=(t == 3))
            Cz_sb = sb.tile([100, 64], F32, tag="Cz_sb")
            nc.vector.tensor_copy(out=Cz_sb[:], in_=Cz_ps[:])
            q1_ps = ps.tile([65, 64], F32, tag="q65", bufs=1, name="q1")
            for t in range(4):
                nc.tensor.matmul(q1_ps[:], KV1[:, t, :], E1[:, t, :],
                                 start=(t == 0), stop=(t == 3))

            # ---- q half: rec broadcast via gpsimd, applied at the C level
            rec1 = sb.tile([1, 64], F32, tag="rec1")
            nc.vector.reciprocal(rec1[:], q1_ps[64:65, :])
            recB_ps = ps.tile([100, 64], F32, tag="qB", bufs=1, name="recB")
            nc.tensor.matmul(recB_ps[:], ones_sb[:], rec1[:],
                             start=True, stop=True)
            recB = sb.tile([100, 64], F32, tag="recB")
            nc.scalar.copy(out=recB[:], in_=recB_ps[:])
            qm = sb.tile([64, 64], BF16, tag="qm")
            nc.vector.tensor_tensor(out=qm[:], in0=q1_ps[0:64, :],
                                    in1=m2[:], op=ALU.mult)
            g1q_ps = ps.tile([128, 4, 64], F32, tag="gf", bufs=2, name="g1q")
            for t in range(4):
                nc.tensor.matmul(g1q_ps[:, t, :],
                                 w2bp[:, t * 128:(t + 1) * 128],
                                 qm[:], start=True, stop=True)
            g1qm = sb.tile([128, 4, 64], BF16, tag="g1qm")
            nc.vector.tensor_tensor(out=g1qm[:], in0=g1q_ps[:], in1=M1W[:],
                                    op=ALU.mult)
            Cq_ps = ps.tile([100, 64], F32, tag="misc", bufs=2)
            for t in range(4):
                nc.tensor.matmul(Cq_ps[:], Scomb[:, t, :], g1qm[:, t, :],
                                 start=(t == 0), stop=(t == 3))
            t1 = sb.tile([100, 64], F32, tag="t1")
            nc.vector.tensor_tensor(out=t1[:], in0=Cq_ps[:], in1=recB[:],
                                    op=ALU.mult)
            C_bf = sb.tile([100, 64], BF16, tag="C_bf")
            nc.vector.tensor_tensor(out=C_bf[:], in0=Cz_sb[:], in1=t1[:],
                                    op=ALU.subtract)

            # ---- e_min dance (all values exact bf16 copies of C entries,
            # so min/compare are consistent under monotone rounding)
            cand_ps = ps.tile([128, 3, 8, 8], F32, tag="gf", bufs=2,
                              name="cand")
            for cc in range(3):
                nc.tensor.matmul(
                    cand_ps[:, cc, :, :].rearrange("k p q -> k (p q)"),
                    CandM[:, cc, :], C_bf[:], start=True, stop=True)
            cand_sb = sb.tile([128, 3, 8, 8], BF16, tag="cand_sb")
            nc.vector.tensor_copy(out=cand_sb[:], in_=cand_ps[:])
            # shift-aligned candidate stack (zero prefill = min-with-0
            # candidate); innermost-axis min-reduce collapses the 8 classes.
            engs = [nc.vector, nc.vector, nc.vector, nc.vector,
                    nc.gpsimd, nc.gpsimd, nc.gpsimd,
                    nc.scalar, nc.scalar]
            for j, k in enumerate([0, 1, 2, 3, 5, 6, 7, 8, 4]):
                cc, kk = divmod(k, 4)
                dp, dq = k // 3 - 1, k % 3 - 1
                i4lo, i4hi = max(0, dp), min(8, 8 + dp)
                j4lo, j4hi = max(0, dq), min(8, 8 + dq)
                srcap = cand_sb[kk * 32:kk * 32 + 16, cc,
                                i4lo - dp:i4hi - dp,
                                j4lo - dq:j4hi - dq, None]
                dstap = cstk[:, i4lo:i4hi, j4lo:j4hi, j:j + 1]
                if engs[j] is nc.scalar:
                    engs[j].copy(out=dstap, in_=srcap)
                else:
                    engs[j].tensor_copy(out=dstap, in_=srcap)
            nc.vector.tensor_reduce(out=eB[:, 2:10, :], in_=cstk[:],
                                    axis=mybir.AxisListType.X, op=ALU.min)
            eBf = eB[:].rearrange("a b c -> a (b c)")
            eW_ps = ps.tile([100, 64], F32, tag="misc", bufs=2)
            for k in range(9):
                dp, dq = k // 3 - 1, k % 3 - 1
                off = 16 + 8 * dp + dq
                nc.tensor.matmul(eW_ps[:], PermB[:, k * 100:(k + 1) * 100],
                                 eBf[:, off:off + 64],
                                 start=(k == 0), stop=(k == 8))
            maskw = sb.tile([100, 64], F32, tag="maskw")
            nc.vector.tensor_tensor(out=maskw[:], in0=C_bf[:], in1=eW_ps[:],
                                    op=ALU.is_le)

            # ---- masked forward: Xm = X * maskw (broadcast over h), bf16
            Xm = sb.tile([100, 3, 64], BF16, tag="Xm")
            mask_b = bass.AP(tensor=maskw[:].tensor, offset=maskw[:].offset,
                             ap=[[64, 100], [0, 3], [1, 64]])
            nc.vector.tensor_tensor(out=Xm[:], in0=X, in1=mask_b, op=ALU.mult)
            u1_ps = ps.tile([128, 4, 64], F32, tag="gf", bufs=2, name="u1")
            for t in range(4):
                for h in range(3):
                    nc.tensor.matmul(u1_ps[:, t, :], W1big[:, h, t, :],
                                     Xm[:, h, :],
                                     start=(h == 0), stop=(h == 2))
            u1m = sb.tile([128, 4, 64], BF16, tag="u1m")
            nc.vector.tensor_tensor(out=u1m[:], in0=u1_ps[:], in1=M1W[:],
                                    op=ALU.mult)
            zm_ps = ps.tile([64, 64], F32, tag="misc", bufs=2)
            for t in range(4):
                nc.tensor.matmul(zm_ps[:], w2fT[:, t, :], u1m[:, t, :],
                                 start=(t == 0), stop=(t == 3))
            z2m = sb.tile([64, 64], BF16, tag="z2m")
            nc.vector.tensor_tensor(out=z2m[:], in0=zm_ps[:], in1=m2[:],
                                    op=ALU.mult)

            # ---- Hopfield #2 -> output [65, 64]: rows 0:64 unnormalized
            # exp-weighted sums, row 64 the per-pq denominators (host divides)
            q2_ps = _hopfield2(nc, sb, ps, z2m[:], KTp, KV1, "2")
            out_sb = sb.tile([65, 64], F32, tag="out_sb")
            nc.vector.tensor_copy(out=out_sb[:], in_=q2_ps[:])
            nc.sync.dma_start(out=out_t[:], in_=out_sb[:])
    nc.compile()
    return nc


def _get_nc(debug=False):
    key = ("nc", debug)
    if key not in _CACHE:
        _CACHE[key] = _build_nc(debug)
    return _CACHE[key]


# ---------------------------------------------------------------- entry point
def kernel(x, w1, b1, w2, b2, K, Vw, _debug=False):
    x = np.asarray(x, np.float32)
    shared = _host_prep(np.asarray(w1, np.float32), np.asarray(b1, np.float32),
                        np.asarray(w2, np.float32), np.asarray(b2, np.float32),
                        np.asarray(K, np.float32), np.asarray(Vw, np.float32))
    w1f, b1c = shared.pop("_w1f"), shared.pop("_b1")
    b2h = shared.pop("_b2")
    bsz = x.shape[0]
    nc = _get_nc(_debug)
    smpls = [_sample_prep(x[b], w1f, b1c, b2h) for b in range(bsz)]
    in_maps = []
    for core in range(N_CORES):
        m = dict(shared)
        m["cv1"], m["cv2"] = smpls[core] if core < bsz else smpls[0]
        in_maps.append(m)
    res = run_bass_kernel_spmd(nc, in_maps, core_ids=list(range(N_CORES)))
    outs = []
    for b in range(bsz):
        q2 = res.results[b]["out"].astype(np.float32)
        outs.append((q2[0:64] / q2[64:65]).reshape(64, 8, 8))
    out = np.stack(outs).astype(np.float32)
    if _debug:
        return out, res
    return out


# revision 15
# speedup vs baseline: 1.2397x; 1.0441x over previous
"""TRN2 Bass kernel for nn_Block1_43542378447225 (v4).

Pipeline (per sample, one NeuronCore; batch=2 -> cores 0/1 do real work):
  conv1 -> relu -> conv2 -> relu (bf16 matmuls, fp32 psum)
  Hopfield #1 in [m,pq] orientation (bf16 scores, ones-column row sums)
  backward split C = C_z - C_q*diag(1/s): the z2 half runs inside Hopfield
  #1's latency shadow; the q half needs no normalization until the very end
  blocked e_min via permutation matmuls (bf16, monotone-rounding-consistent)
  masked patch forward (bf16) -> z2_masked -> Hopfield #2 -> output

Host precomputes im2col patches, weight layouts, K@Vw (+ones column), packed
into 6 DMA blobs; the device does all matmuls/exp/masking. The final softmax
normalization division (row 64 of the output carries the per-pq denominators)
runs on host.

Layout conventions:
  pq = p*8+q (64 output positions), uv = u*10+v (100 composite-window offsets)
  chunk t = conv2 kernel row kr, a = conv2 kernel col ks
  kc = t*128 + a*32 + c1 (hidden index, 4 chunks of 128 partitions)
  Hopfield memory chunks: m-chunk t = rows t*128:(t+1)*128 of the 512 codebook
"""
import numpy as np

import concourse.bass as bass
import concourse.bacc as bacc
import concourse.mybir as mybir
import concourse.tile as tile
from concourse.bass_utils import run_bass_kernel_spmd

F32 = mybir.dt.float32
BF16 = mybir.dt.bfloat16
AF = mybir.ActivationFunctionType
ALU = mybir.AluOpType

N_CORES = 8
BETA = 0.125  # 1/sqrt(64)

_CACHE = {}


# ---------------------------------------------------------------- host prep
def _bf16(a):
    """Round-to-nearest-even fp32 -> bf16 (ml_dtypes array for PJRT binding)."""
    import ml_dtypes
    return np.ascontiguousarray(a, np.float32).astype(ml_dtypes.bfloat16)


def _build_scomb_w1big(w1):
    w1s = w1.sum(axis=1)
    Scomb = np.zeros((4, 32, 4, 100), np.float32)  # [a, c1, t, uv]
    W1big = np.zeros((100, 3, 4, 4, 32), np.float32)  # [uv, h, t, a, c1]
    for t in range(4):
        for a in range(4):
            for u in range(10):
                ki = u - 2 * t
                if not (0 <= ki < 4):
                    continue
                for v in range(10):
                    kj = v - 2 * a
                    if not (0 <= kj < 4):
                        continue
                    Scomb[a, :, t, u * 10 + v] = w1s[:, ki, kj]
                    W1big[u * 10 + v, :, t, a, :] = w1[:, :, ki, kj].T
    # partition index = a*32+c1 -> merge (a, c1); free = t*100+uv
    Scomb = Scomb.reshape(128, 400)
    W1big = W1big.reshape(100, 1536)
    return Scomb, W1big


def _host_prep(w1, b1, w2, b2, K, Vw):
    Scomb, W1big = _build_scomb_w1big(w1)
    PermF = np.zeros((100, 9, 16), np.float32)
    for k in range(9):
        dp, dq = k // 3 - 1, k % 3 - 1
        for im in range(4):
            u = 4 * dp + im + 3
            if not (0 <= u < 10):
                continue
            for jm in range(4):
                v = 4 * dq + jm + 3
                if not (0 <= v < 10):
                    continue
                PermF[u * 10 + v, k, im * 4 + jm] = 1.0
    CandM = np.zeros((100, 3, 128), np.float32)
    for k in range(9):
        cc, kk = divmod(k, 4)
        CandM[:, cc, kk * 32:kk * 32 + 16] = PermF[:, k, :]
    PermB = np.transpose(PermF, (2, 1, 0)).reshape(16, 900)

    w2fT = np.transpose(w2, (3, 1, 2, 0)).reshape(128, 256)
    w2b = 2.0 * np.transpose(w2, (0, 2, 3, 1)).reshape(64, 512)
    KT = np.ascontiguousarray(K.T)                       # [64, 512]
    KVc = (K @ Vw).reshape(4, 128, 64)                   # m-chunks

    # wB64 [64, 1024] bf16: KT | w2b (both contract over c=64, base part 0)
    wB64 = np.zeros((64, 1024), np.float32)
    wB64[:, 0:512] = KT
    wB64[:, 512:1024] = w2b
    # wBF [128, 916] bf16: KV1 (4x65) | Scomb | w2fT
    wBF = np.zeros((128, 916), np.float32)
    for t in range(4):
        wBF[:, t * 65:t * 65 + 64] = KVc[t]
        wBF[:, t * 65 + 64] = 1.0
    wBF[:, 260:660] = Scomb
    wBF[:, 660:916] = w2fT

    # wCW [100, 1920] bf16: W1big | CandM
    wCW = np.zeros((100, 1920), np.float32)
    wCW[:, 0:1536] = W1big
    wCW[:, 1536:1920] = CandM.reshape(100, 384)

    return {"wB64": _bf16(wB64), "wBF": _bf16(wBF),
            "wCW": _bf16(wCW), "wPB": _bf16(PermB),
            "_b2": b2,
            "_w1f": np.ascontiguousarray(np.transpose(w1, (2, 3, 1, 0)).reshape(48, 32)),
            "_b1": np.ascontiguousarray(b1[:, None])}


def _sample_prep(x_s, w1f, b1c, b2):
    xp1 = np.pad(x_s, ((0, 0), (1, 1), (1, 1)))
    xp3 = np.pad(x_s, ((0, 0), (3, 3), (3, 3)))
    P1 = np.zeros((4, 4, 3, 16, 16), np.float32)
    for kr in range(4):
        for ks in range(4):
            P1[kr, ks] = xp1[:, kr:kr + 32:2, ks:ks + 32:2][:, :16, :16]
    X = np.zeros((10, 10, 3, 8, 8), np.float32)
    for u in range(10):
        for v in range(10):
            X[u, v] = xp3[:, u:u + 32:4, v:v + 32:4][:, :8, :8]
    # cv1 [48, 289] bf16: P1 | w1f | b1  (smallest, first -> earliest conv1)
    cv1 = np.zeros((48, 289), np.float32)
    cv1[:, 0:256] = P1.reshape(48, 256)
    cv1[:, 256:288] = w1f
    cv1[0:32, 288:289] = b1c
    # cv2 [100, 193] fp32: b2 | X
    cv2 = np.zeros((100, 193), np.float32)
    cv2[0:64, 0:1] = b2[:, None]
    cv2[:, 1:193] = X.reshape(100, 192)
    return _bf16(cv1), cv2


# ---------------------------------------------------------------- device build
def _hopfield2(nc, sb, ps, z_bf, KTp, KV1, tag):
    """z_bf [64(c), 64(pq)] bf16 -> q_ps [65, 64] PSUM fp32.
    Scores computed directly in [m, pq] orientation (no transposes); the
    ones column folded into KV1 puts the per-pq exp-sum in row 64."""
    S_ps = ps.tile([128, 4, 64], F32, tag="S", bufs=1, name=f"S{tag}")
    for t in range(4):
        nc.tensor.matmul(S_ps[:, t, :], KTp[:, t * 128:(t + 1) * 128], z_bf,
                         start=True, stop=True)
    E = sb.tile([128, 4, 64], BF16, tag=f"E{tag}", name=f"E{tag}")
    nc.scalar.activation(out=E[:], in_=S_ps[:], func=AF.Exp,
                         bias=0.0, scale=BETA)
    q_ps = ps.tile([65, 64], F32, tag="q65", bufs=1, name=f"q{tag}")
    for t in range(4):
        nc.tensor.matmul(q_ps[:], KV1[:, t, :], E[:, t, :],
                         start=(t == 0), stop=(t == 3))
    return q_ps


def _build_nc(debug=False):
    nc = bacc.Bacc("TRN2", target_bir_lowering=False, debug=False,
                   num_devices=N_CORES)
    d_cv1 = nc.dram_tensor("cv1", [48, 289], BF16, kind="ExternalInput")
    d_cv2 = nc.dram_tensor("cv2", [100, 193], F32, kind="ExternalInput")
    d_wB64 = nc.dram_tensor("wB64", [64, 1024], BF16, kind="ExternalInput")
    d_wBF = nc.dram_tensor("wBF", [128, 916], BF16, kind="ExternalInput")
    d_wCW = nc.dram_tensor("wCW", [100, 1920], BF16, kind="ExternalInput")
    d_wPB = nc.dram_tensor("wPB", [16, 900], BF16, kind="ExternalInput")
    out_t = nc.dram_tensor("out", [65, 64], F32, kind="ExternalOutput")

    with tile.TileContext(nc) as tc:
        with tc.tile_pool(name="sb", bufs=1) as sb, \
             tc.tile_pool(name="ps", bufs=1, space="PSUM") as ps:
            # ---- PE warm-up out of the cold p-state before conv1 arrives.
            warm = sb.tile([2, 8], F32, tag="warm")
            nc.gpsimd.memset(warm[:], 0.0)
            for w_ in range(3):
                warm_ps = ps.tile([8, 8], F32, tag="qB", bufs=1,
                                  name=f"warm{w_}")
                nc.tensor.matmul(warm_ps[:], warm[0:2, :], warm[0:2, :],
                                 start=True, stop=True)

            # ---- loads (SP queue: need order; Pool queue: PermB via SWDGE;
            # Act stays free of DMA configs so the first relu isn't delayed)
            cv1 = sb.tile([48, 289], BF16, tag="cv1")
            nc.sync.dma_start(out=cv1[:], in_=d_cv1[:])
            cv2 = sb.tile([100, 193], F32, tag="cv2")
            nc.sync.dma_start(out=cv2[:], in_=d_cv2[:])
            wB64 = sb.tile([64, 1024], BF16, tag="wB64")
            nc.sync.dma_start(out=wB64[:], in_=d_wB64[:])
            wBF = sb.tile([128, 916], BF16, tag="wBF")
            nc.sync.dma_start(out=wBF[:], in_=d_wBF[:])
            wCW = sb.tile([100, 1920], BF16, tag="wCW")
            nc.sync.dma_start(out=wCW[:], in_=d_wCW[:])
            wPB = sb.tile([16, 900], BF16, tag="wPB")
            nc.gpsimd.dma_start(out=wPB[:], in_=d_wPB[:])

            P1 = cv1[0:48, 0:256]
            w1f = cv1[0:48, 256:288]
            b1 = cv1[0:32, 288:289]
            b2 = cv2[0:64, 0:1]
            X = cv2[0:100, 1:193].rearrange("u (h q) -> u h q", h=3)
            KTp = wB64[0:64, 0:512]
            w2bp = wB64[0:64, 512:1024]
            KV1 = wBF[:, 0:260].rearrange("k (t u) -> k t u", t=4)
            Scomb = wBF[:, 260:660].rearrange("k (t u) -> k t u", t=4)
            w2fT = wBF[:, 660:916].rearrange("k (t c) -> k t c", t=4)
            W1big = wCW[0:100, 0:1536].rearrange("u (h t k) -> u h t k",
                                                 h=3, t=4)
            CandM = wCW[0:100, 1536:1920].rearrange("u (c k) -> u c k", c=3)
            PermB = wPB[0:16, :]

            # dummy activation: binds the hoisted act-table load to the
            # DMA-wait window instead of the first data-dependent relu
            warm_act = sb.tile([2, 8], F32, tag="warm_act")
            nc.scalar.activation(out=warm_act[:], in_=warm[:],
                                 func=AF.Relu, bias=0.0, scale=1.0)

            # early memsets fill the initial DMA-wait window
            cstk = sb.tile([16, 8, 8, 10], BF16, tag="cstk")
            nc.gpsimd.memset(cstk[:], 0.0)
            a1p = sb.tile([32, 18, 18], BF16, tag="a1p")
            nc.gpsimd.memset(a1p[:], 0.0)
            eB = sb.tile([16, 12, 8], BF16, tag="eB")
            nc.vector.memset(eB[:], 0.0)
            ones_sb = sb.tile([1, 100], F32, tag="ones")
            nc.vector.memset(ones_sb[:], 1.0)

            # ---- conv1 + relu into padded a1p [32, 18, 18] (bf16)
            a1_ps = ps.tile([32, 256], F32, tag="misc", bufs=2)
            nc.tensor.matmul(a1_ps[:], w1f, P1, start=True, stop=True)
            nc.scalar.activation(
                out=a1p[:, 1:17, 1:17],
                in_=a1_ps[:].rearrange("c (p q) -> c p q", p=16),
                func=AF.Relu, bias=b1, scale=1.0)

            # ---- P2 im2col: P2[a*32+c1, t, p, q] = a1p[c1, 2p+t, 2q+a]
            P2 = sb.tile([128, 4, 8, 8], BF16, tag="P2")
            a1p_ap = a1p[:]
            for a in range(4):
                src = bass.AP(
                    tensor=a1p_ap.tensor,
                    offset=a1p_ap.offset + a,
                    ap=[[324, 32], [18, 4], [36, 8], [2, 8]])
                eng = (nc.vector, nc.scalar, nc.vector, nc.gpsimd)[a]
                if eng is nc.scalar:
                    eng.copy(out=P2[a * 32:(a + 1) * 32, :, :, :], in_=src)
                else:
                    eng.tensor_copy(out=P2[a * 32:(a + 1) * 32, :, :, :],
                                    in_=src)
            P2f = P2[:].rearrange("k t p q -> k t (p q)")
            M1W = sb.tile([128, 4, 64], F32, tag="M1W")
            nc.vector.tensor_scalar(out=M1W[:], in0=P2f, scalar1=0.0,
                                    scalar2=None, op0=ALU.not_equal)

            # ---- conv2 -> z2 (bf16 matmul); z_bf on Act, z2 fp32 on DVE
            z2_ps = ps.tile([64, 64], F32, tag="misc", bufs=2)
            for t in range(4):
                nc.tensor.matmul(z2_ps[:], w2fT[:, t, :], P2f[:, t, :],
                                 start=(t == 0), stop=(t == 3))
            z_bf = sb.tile([64, 64], BF16, tag="z_bf")
            nc.scalar.activation(out=z_bf[:], in_=z2_ps[:],
                                 func=AF.Relu, bias=b2, scale=1.0)
            z2 = sb.tile([64, 64], F32, tag="z2")
            nc.vector.tensor_scalar(out=z2[:], in0=z2_ps[:], scalar1=b2,
                                    scalar2=0.0, op0=ALU.add, op1=ALU.max)
            m2 = sb.tile([64, 64], F32, tag="m2")
            nc.vector.tensor_scalar(out=m2[:], in0=z2[:], scalar1=0.0,
                                    scalar2=None, op0=ALU.not_equal)

            # ---- Hopfield #1 (PE also squeezes the C_z half of the
            # backward into the latency shadow: C = C_z - C_q * diag(1/s))
            S1_ps = ps.tile([128, 4, 64], F32, tag="S", bufs=1, name="S1")
            for t in range(4):
                nc.tensor.matmul(S1_ps[:, t, :], KTp[:, t * 128:(t + 1) * 128],
                                 z_bf[:], start=True, stop=True)
            # z2 half of the backward (rhs z_bf, no Hopfield dependency)
            g1z_ps = ps.tile([128, 4, 64], F32, tag="gf", bufs=2, name="g1z")
            for t in range(4):
                nc.tensor.matmul(g1z_ps[:, t, :],
                                 w2bp[:, t * 128:(t + 1) * 128],
                                 z_bf[:], start=True, stop=True)
            E1 = sb.tile([128, 4, 64], BF16, tag="E1", name="E1")
            nc.scalar.activation(out=E1[:], in_=S1_ps[:], func=AF.Exp,
                                 bias=0.0, scale=BETA)
            g1zm = sb.tile([128, 4, 64], BF16, tag="g1zm")
            nc.vector.tensor_tensor(out=g1zm[:], in0=g1z_ps[:], in1=M1W[:],
                                    op=ALU.mult)
            Cz_ps = ps.tile([100, 64], F32, tag="misc", bufs=2)
            for t in range(4):
                nc.tensor.matmul(Cz_ps[:], Scomb[:, t, :], g1zm[:, t, :],
                                 start=(t == 0), stop=(t == 3))
            Cz_sb = sb.tile([100, 64], F32, tag="Cz_sb")
            nc.vector.tensor_copy(out=Cz_sb[:], in_=Cz_ps[:])
            q1_ps = ps.tile([65, 64], F32, tag="q65", bufs=1, name="q1")
            for t in range(4):
                nc.tensor.matmul(q1_ps[:], KV1[:, t, :], E1[:, t, :],
                                 start=(t == 0), stop=(t == 3))

            # ---- q half: rec broadcast via gpsimd, applied at the C level
            rec1 = sb.tile([1, 64], F32, tag="rec1")
            nc.vector.reciprocal(rec1[:], q1_ps[64:65, :])
            recB_ps = ps.tile([100, 64], F32, tag="qB", bufs=1, name="recB")
            nc.tensor.matmul(recB_ps[:], ones_sb[:], rec1[:],
                             start=True, stop=True)
            recB = sb.tile([100, 64], F32, tag="recB")
            nc.scalar.copy(out=recB[:], in_=recB_ps[:])
            qm = sb.tile([64, 64], BF16, tag="qm")
            nc.vector.tensor_tensor(out=qm[:], in0=q1_ps[0:64, :],
                                    in1=m2[:], op=ALU.mult)
            g1q_ps = ps.tile([128, 4, 64], F32, tag="gf", bufs=2, name="g1q")
            for t in range(4):
                nc.tensor.matmul(g1q_ps[:, t, :],
                                 w2bp[:, t * 128:(t + 1) * 128],
                                 qm[:], start=True, stop=True)
            g1qm = sb.tile([128, 4, 64], BF16, tag="g1qm")
            nc.vector.tensor_tensor(out=g1qm[:], in0=g1q_ps[:], in1=M1W[:],
                                    op=ALU.mult)
            Cq_ps = ps.tile([100, 64], F32, tag="misc", bufs=2)
            for t in range(4):
                nc.tensor.matmul(Cq_ps[:], Scomb[:, t, :], g1qm[:, t, :],
                                 start=(t == 0), stop=(t == 3))
            t1 = sb.tile([100, 64], F32, tag="t1")
            nc.vector.tensor_tensor(out=t1[:], in0=Cq_ps[:], in1=recB[:],
                                    op=ALU.mult)
            C_bf = sb.tile([100, 64], BF16, tag="C_bf")
            nc.vector.tensor_tensor(out=C_bf[:], in0=Cz_sb[:], in1=t1[:],
                                    op=ALU.subtract)

            # ---- e_min dance (all values exact bf16 copies of C entries,
            # so min/compare are consistent under monotone rounding)
            cand_ps = ps.tile([128, 3, 8, 8], F32, tag="gf", bufs=2,
                              name="cand")
            for cc in range(3):
                nc.tensor.matmul(
                    cand_ps[:, cc, :, :].rearrange("k p q -> k (p q)"),
                    CandM[:, cc, :], C_bf[:], start=True, stop=True)
            cand_sb = sb.tile([128, 3, 8, 8], BF16, tag="cand_sb")
            nc.vector.tensor_copy(out=cand_sb[:], in_=cand_ps[:])
            # shift-aligned candidate stack (zero prefill = min-with-0
            # candidate); innermost-axis min-reduce collapses the 8 classes.
            engs = [nc.vector, nc.vector, nc.vector, nc.vector,
                    nc.gpsimd, nc.gpsimd, nc.gpsimd,
                    nc.scalar, nc.scalar]
            for j, k in enumerate([0, 1, 2, 3, 5, 6, 7, 8, 4]):
                cc, kk = divmod(k, 4)
                dp, dq = k // 3 - 1, k % 3 - 1
                i4lo, i4hi = max(0, dp), min(8, 8 + dp)
                j4lo, j4hi = max(0, dq), min(8, 8 + dq)
                srcap = cand_sb[kk * 32:kk * 32 + 16, cc,
                                i4lo - dp:i4hi - dp,
                                j4lo - dq:j4hi - dq, None]
                dstap = cstk[:, i4lo:i4hi, j4lo:j4hi, j:j + 1]
                if engs[j] is nc.scalar:
                    engs[j].copy(out=dstap, in_=srcap)
                else:
                    engs[j].tensor_copy(out=dstap, in_=srcap)
            nc.vector.tensor_reduce(out=eB[:, 2:10, :], in_=cstk[:],
                                    axis=mybir.AxisListType.X, op=ALU.min)
            eBf = eB[:].rearrange("a b c -> a (b c)")
            eW_ps = ps.tile([100, 64], F32, tag="misc", bufs=2)
            for k in range(9):
                dp, dq = k // 3 - 1, k % 3 - 1
                off = 16 + 8 * dp + dq
                nc.tensor.matmul(eW_ps[:], PermB[:, k * 100:(k + 1) * 100],
                                 eBf[:, off:off + 64],
                                 start=(k == 0), stop=(k == 8))
            maskw = sb.tile([100, 64], F32, tag="maskw")
            nc.vector.tensor_tensor(out=maskw[:], in0=C_bf[:], in1=eW_ps[:],
                                    op=ALU.is_le)

            # ---- masked forward: Xm = X * maskw (broadcast over h), bf16
            Xm = sb.tile([100, 3, 64], BF16, tag="Xm")
            mask_b = bass.AP(tensor=maskw[:].tensor, offset=maskw[:].offset,
                             ap=[[64, 100], [0, 3], [1, 64]])
            nc.vector.tensor_tensor(out=Xm[:], in0=X, in1=mask_b, op=ALU.mult)
            u1_ps = ps.tile([128, 4, 64], F32, tag="gf", bufs=2, name="u1")
            for t in range(4):
                for h in range(3):
                    nc.tensor.matmul(u1_ps[:, t, :], W1big[:, h, t, :],
                                     Xm[:, h, :],
                                     start=(h == 0), stop=(h == 2))
            u1m = sb.tile([128, 4, 64], BF16, tag="u1m")
            nc.vector.tensor_tensor(out=u1m[:], in0=u1_ps[:], in1=M1W[:],
                                    op=ALU.mult)
            zm_ps = ps.tile([64, 64], F32, tag="misc", bufs=2)
            for t in range(4):
                nc.tensor.matmul(zm_ps[:], w2fT[:, t, :], u1m[:, t, :],
                                 start=(t == 0), stop=(t == 3))
            z2m = sb.tile([64, 64], BF16, tag="z2m")
            nc.vector.tensor_tensor(out=z2m[:], in0=zm_ps[:], in1=m2[:],
                                    op=ALU.mult)

            # ---- Hopfield #2 -> output [65, 64]: rows 0:64 unnormalized
            # exp-weighted sums, row 64 the per-pq denominators (host divides)
            q2_ps = _hopfield2(nc, sb, ps, z2m[:], KTp, KV1, "2")
            out_sb = sb.tile([65, 64], F32, tag="out_sb")
            nc.vector.tensor_copy(out=out_sb[:], in_=q2_ps[:])
            nc.sync.dma_start(out=out_t[:], in_=out_sb[:])
    nc.compile()
    return nc


def _get_nc(debug=False):
    key = ("nc", debug)
    if key not in _CACHE:
        _CACHE[key] = _build_nc(debug)
    return _CACHE[key]


# ---------------------------------------------------------------- entry point
def kernel(x, w1, b1, w2, b2, K, Vw, _debug=False):
    x = np.asarray(x, np.float32)
    shared = _host_prep(np.asarray(w1, np.float32), np.asarray(b1, np.float32),
                        np.asarray(w2, np.float32), np.asarray(b2, np.float32),
                        np.asarray(K, np.float32), np.asarray(Vw, np.float32))
    w1f, b1c = shared.pop("_w1f"), shared.pop("_b1")
    b2h = shared.pop("_b2")
    bsz = x.shape[0]
    nc = _get_nc(_debug)
    smpls = [_sample_prep(x[b], w1f, b1c, b2h) for b in range(bsz)]
    in_maps = []
    for core in range(N_CORES):
        m = dict(shared)
        m["cv1"], m["cv2"] = smpls[core] if core < bsz else smpls[0]
        in_maps.append(m)
    res = run_bass_kernel_spmd(nc, in_maps, core_ids=list(range(N_CORES)))
    outs = []
    for b in range(bsz):
        q2 = res.results[b]["out"].astype(np.float32)
        outs.append((q2[0:64] / q2[64:65]).reshape(64, 8, 8))
    out = np.stack(outs).astype(np.float32)
    if _debug:
        return out, res
    return out


# revision 16
# speedup vs baseline: 1.2834x; 1.0352x over previous
"""TRN2 Bass kernel for nn_Block1_43542378447225 (v4).

Pipeline (per sample, one NeuronCore; batch=2 -> cores 0/1 do real work):
  conv1 -> relu -> conv2 -> relu (bf16 matmuls, fp32 psum)
  Hopfield #1 in [m,pq] orientation (bf16 scores, ones-column row sums)
  backward split C = C_z - C_q*diag(1/s): the z2 half runs inside Hopfield
  #1's latency shadow; the q half needs no normalization until the very end
  blocked e_min via permutation matmuls (bf16, monotone-rounding-consistent)
  masked patch forward (bf16) -> z2_masked -> Hopfield #2 -> output

Host precomputes im2col patches, weight layouts, K@Vw (+ones column), packed
into 6 DMA blobs; the device does all matmuls/exp/masking. The final softmax
normalization division (row 64 of the output carries the per-pq denominators)
runs on host.

Layout conventions:
  pq = p*8+q (64 output positions), uv = u*10+v (100 composite-window offsets)
  chunk t = conv2 kernel row kr, a = conv2 kernel col ks
  kc = t*128 + a*32 + c1 (hidden index, 4 chunks of 128 partitions)
  Hopfield memory chunks: m-chunk t = rows t*128:(t+1)*128 of the 512 codebook
"""
import numpy as np

import concourse.bass as bass
import concourse.bacc as bacc
import concourse.mybir as mybir
import concourse.tile as tile
from concourse.bass_utils import run_bass_kernel_spmd

F32 = mybir.dt.float32
BF16 = mybir.dt.bfloat16
AF = mybir.ActivationFunctionType
ALU = mybir.AluOpType

N_CORES = 8
BETA = 0.125  # 1/sqrt(64)

_CACHE = {}


# ---------------------------------------------------------------- host prep
def _bf16(a):
    """Round-to-nearest-even fp32 -> bf16 (ml_dtypes array for PJRT binding)."""
    import ml_dtypes
    return np.ascontiguousarray(a, np.float32).astype(ml_dtypes.bfloat16)


def _build_scomb_w1big(w1):
    w1s = w1.sum(axis=1)
    Scomb = np.zeros((4, 32, 4, 100), np.float32)  # [a, c1, t, uv]
    W1big = np.zeros((100, 3, 4, 4, 32), np.float32)  # [uv, h, t, a, c1]
    for t in range(4):
        for a in range(4):
            for u in range(10):
                ki = u - 2 * t
                if not (0 <= ki < 4):
                    continue
                for v in range(10):
                    kj = v - 2 * a
                    if not (0 <= kj < 4):
                        continue
                    Scomb[a, :, t, u * 10 + v] = w1s[:, ki, kj]
                    W1big[u * 10 + v, :, t, a, :] = w1[:, :, ki, kj].T
    # partition index = a*32+c1 -> merge (a, c1); free = t*100+uv
    Scomb = Scomb.reshape(128, 400)
    W1big = W1big.reshape(100, 1536)
    return Scomb, W1big


def _host_prep(w1, b1, w2, b2, K, Vw):
    Scomb, W1big = _build_scomb_w1big(w1)
    PermF = np.zeros((100, 9, 16), np.float32)
    for k in range(9):
        dp, dq = k // 3 - 1, k % 3 - 1
        for im in range(4):
            u = 4 * dp + im + 3
            if not (0 <= u < 10):
                continue
            for jm in range(4):
                v = 4 * dq + jm + 3
                if not (0 <= v < 10):
                    continue
                PermF[u * 10 + v, k, im * 4 + jm] = 1.0
    CandM = np.zeros((100, 3, 128), np.float32)
    for k in range(9):
        cc, kk = divmod(k, 4)
        CandM[:, cc, kk * 32:kk * 32 + 16] = PermF[:, k, :]
    PermB = np.transpose(PermF, (2, 1, 0)).reshape(16, 900)

    w2fT = np.transpose(w2, (3, 1, 2, 0)).reshape(128, 256)
    w2b = 2.0 * np.transpose(w2, (0, 2, 3, 1)).reshape(64, 512)
    KT = np.ascontiguousarray(K.T)                       # [64, 512]
    KVc = (K @ Vw).reshape(4, 128, 64)                   # m-chunks

    # wB64 [64, 1024] bf16: KT | w2b (both contract over c=64, base part 0)
    wB64 = np.zeros((64, 1024), np.float32)
    wB64[:, 0:512] = KT
    wB64[:, 512:1024] = w2b
    # wBF [128, 916] bf16: KV1 (4x65) | Scomb | w2fT
    wBF = np.zeros((128, 916), np.float32)
    for t in range(4):
        wBF[:, t * 65:t * 65 + 64] = KVc[t]
        wBF[:, t * 65 + 64] = 1.0
    wBF[:, 260:660] = Scomb
    wBF[:, 660:916] = w2fT

    # wCW [100, 1920] bf16: W1big | CandM
    wCW = np.zeros((100, 1920), np.float32)
    wCW[:, 0:1536] = W1big
    wCW[:, 1536:1920] = CandM.reshape(100, 384)

    return {"wB64": _bf16(wB64), "wBF": _bf16(wBF),
            "wCW": _bf16(wCW), "wPB": _bf16(PermB),
            "_b2": b2,
            "_w1f": np.ascontiguousarray(np.transpose(w1, (2, 3, 1, 0)).reshape(48, 32)),
            "_b1": np.ascontiguousarray(b1[:, None])}


def _sample_prep(x_s, w1f, b1c, b2):
    xp1 = np.pad(x_s, ((0, 0), (1, 1), (1, 1)))
    xp3 = np.pad(x_s, ((0, 0), (3, 3), (3, 3)))
    P1 = np.zeros((4, 4, 3, 16, 16), np.float32)
    for kr in range(4):
        for ks in range(4):
            P1[kr, ks] = xp1[:, kr:kr + 32:2, ks:ks + 32:2][:, :16, :16]
    X = np.zeros((10, 10, 3, 8, 8), np.float32)
    for u in range(10):
        for v in range(10):
            X[u, v] = xp3[:, u:u + 32:4, v:v + 32:4][:, :8, :8]
    # cv1 [48, 289] bf16: P1 | w1f | b1  (smallest, first -> earliest conv1)
    cv1 = np.zeros((48, 289), np.float32)
    cv1[:, 0:256] = P1.reshape(48, 256)
    cv1[:, 256:288] = w1f
    cv1[0:32, 288:289] = b1c
    # cv2 [100, 193] fp32: b2 | X
    cv2 = np.zeros((100, 193), np.float32)
    cv2[0:64, 0:1] = b2[:, None]
    cv2[:, 1:193] = X.reshape(100, 192)
    return _bf16(cv1), cv2


# ---------------------------------------------------------------- device build
def _hopfield2(nc, sb, ps, z_bf, KTp, KV1, tag):
    """z_bf [64(c), 64(pq)] bf16 -> q_ps [65, 64] PSUM fp32.
    Scores computed directly in [m, pq] orientation (no transposes); the
    ones column folded into KV1 puts the per-pq exp-sum in row 64."""
    S_ps = ps.tile([128, 4, 64], F32, tag="S", bufs=1, name=f"S{tag}")
    for t in range(4):
        nc.tensor.matmul(S_ps[:, t, :], KTp[:, t * 128:(t + 1) * 128], z_bf,
                         start=True, stop=True)
    E = sb.tile([128, 4, 64], BF16, tag=f"E{tag}", name=f"E{tag}")
    nc.scalar.activation(out=E[:], in_=S_ps[:], func=AF.Exp,
                         bias=0.0, scale=BETA)
    q_ps = ps.tile([65, 64], F32, tag="q65", bufs=1, name=f"q{tag}")
    for t in range(4):
        nc.tensor.matmul(q_ps[:], KV1[:, t, :], E[:, t, :],
                         start=(t == 0), stop=(t == 3))
    return q_ps


def _build_nc(debug=False):
    nc = bacc.Bacc("TRN2", target_bir_lowering=False, debug=False,
                   num_devices=N_CORES)
    d_cv1 = nc.dram_tensor("cv1", [48, 289], BF16, kind="ExternalInput")
    d_cv2 = nc.dram_tensor("cv2", [100, 193], F32, kind="ExternalInput")
    d_wB64 = nc.dram_tensor("wB64", [64, 1024], BF16, kind="ExternalInput")
    d_wBF = nc.dram_tensor("wBF", [128, 916], BF16, kind="ExternalInput")
    d_wCW = nc.dram_tensor("wCW", [100, 1920], BF16, kind="ExternalInput")
    d_wPB = nc.dram_tensor("wPB", [16, 900], BF16, kind="ExternalInput")
    out_t = nc.dram_tensor("out", [65, 64], F32, kind="ExternalOutput")

    with tile.TileContext(nc) as tc:
        with tc.tile_pool(name="sb", bufs=1) as sb, \
             tc.tile_pool(name="ps", bufs=1, space="PSUM") as ps:
            # ---- PE warm-up out of the cold p-state before conv1 arrives.
            warm = sb.tile([2, 8], F32, tag="warm")
            nc.gpsimd.memset(warm[:], 0.0)
            for w_ in range(3):
                warm_ps = ps.tile([8, 8], F32, tag="qB", bufs=1,
                                  name=f"warm{w_}")
                nc.tensor.matmul(warm_ps[:], warm[0:2, :], warm[0:2, :],
                                 start=True, stop=True)

            # ---- loads (SP queue: need order; Pool queue: PermB via SWDGE;
            # Act stays free of DMA configs so the first relu isn't delayed)
            cv1 = sb.tile([48, 289], BF16, tag="cv1")
            nc.sync.dma_start(out=cv1[:], in_=d_cv1[:])
            wBF = sb.tile([128, 916], BF16, tag="wBF")
            nc.sync.dma_start(out=wBF[:], in_=d_wBF[:])
            wB64 = sb.tile([64, 1024], BF16, tag="wB64")
            nc.sync.dma_start(out=wB64[:], in_=d_wB64[:])
            cv2 = sb.tile([100, 193], F32, tag="cv2")
            nc.sync.dma_start(out=cv2[:], in_=d_cv2[:])
            wCW = sb.tile([100, 1920], BF16, tag="wCW")
            nc.sync.dma_start(out=wCW[:], in_=d_wCW[:])
            wPB = sb.tile([16, 900], BF16, tag="wPB")
            nc.gpsimd.dma_start(out=wPB[:], in_=d_wPB[:])

            P1 = cv1[0:48, 0:256]
            w1f = cv1[0:48, 256:288]
            b1 = cv1[0:32, 288:289]
            b2 = cv2[0:64, 0:1]
            X = cv2[0:100, 1:193].rearrange("u (h q) -> u h q", h=3)
            KTp = wB64[0:64, 0:512]
            w2bp = wB64[0:64, 512:1024]
            KV1 = wBF[:, 0:260].rearrange("k (t u) -> k t u", t=4)
            Scomb = wBF[:, 260:660].rearrange("k (t u) -> k t u", t=4)
            w2fT = wBF[:, 660:916].rearrange("k (t c) -> k t c", t=4)
            W1big = wCW[0:100, 0:1536].rearrange("u (h t k) -> u h t k",
                                                 h=3, t=4)
            CandM = wCW[0:100, 1536:1920].rearrange("u (c k) -> u c k", c=3)
            PermB = wPB[0:16, :]

            # dummy activation: binds the hoisted act-table load to the
            # DMA-wait window instead of the first data-dependent relu
            warm_act = sb.tile([2, 8], F32, tag="warm_act")
            nc.scalar.activation(out=warm_act[:], in_=warm[:],
                                 func=AF.Relu, bias=0.0, scale=1.0)

            # early memsets fill the initial DMA-wait window
            cstk = sb.tile([16, 8, 8, 10], BF16, tag="cstk")
            nc.gpsimd.memset(cstk[:], 0.0)
            a1p = sb.tile([32, 18, 18], BF16, tag="a1p")
            nc.gpsimd.memset(a1p[:], 0.0)
            eB = sb.tile([16, 12, 8], BF16, tag="eB")
            nc.vector.memset(eB[:], 0.0)
            ones_sb = sb.tile([1, 100], F32, tag="ones")
            nc.vector.memset(ones_sb[:], 1.0)

            # ---- conv1 + relu into padded a1p [32, 18, 18] (bf16)
            a1_ps = ps.tile([32, 256], F32, tag="misc", bufs=2)
            nc.tensor.matmul(a1_ps[:], w1f, P1, start=True, stop=True)
            nc.scalar.activation(
                out=a1p[:, 1:17, 1:17],
                in_=a1_ps[:].rearrange("c (p q) -> c p q", p=16),
                func=AF.Relu, bias=b1, scale=1.0)

            # ---- P2 im2col: P2[a*32+c1, t, p, q] = a1p[c1, 2p+t, 2q+a]
            P2 = sb.tile([128, 4, 8, 8], BF16, tag="P2")
            a1p_ap = a1p[:]
            for a in range(4):
                src = bass.AP(
                    tensor=a1p_ap.tensor,
                    offset=a1p_ap.offset + a,
                    ap=[[324, 32], [18, 4], [36, 8], [2, 8]])
                eng = (nc.vector, nc.scalar, nc.vector, nc.gpsimd)[a]
                if eng is nc.scalar:
                    eng.copy(out=P2[a * 32:(a + 1) * 32, :, :, :], in_=src)
                else:
                    eng.tensor_copy(out=P2[a * 32:(a + 1) * 32, :, :, :],
                                    in_=src)
            P2f = P2[:].rearrange("k t p q -> k t (p q)")
            M1W = sb.tile([128, 4, 64], F32, tag="M1W")
            nc.vector.tensor_scalar(out=M1W[:], in0=P2f, scalar1=0.0,
                                    scalar2=None, op0=ALU.not_equal)

            # ---- conv2 -> z2 (bf16 matmul); z_bf on Act, z2 fp32 on DVE
            z2_ps = ps.tile([64, 64], F32, tag="misc", bufs=2)
            for t in range(4):
                nc.tensor.matmul(z2_ps[:], w2fT[:, t, :], P2f[:, t, :],
                                 start=(t == 0), stop=(t == 3))
            z_bf = sb.tile([64, 64], BF16, tag="z_bf")
            nc.scalar.activation(out=z_bf[:], in_=z2_ps[:],
                                 func=AF.Relu, bias=b2, scale=1.0)
            z2 = sb.tile([64, 64], F32, tag="z2")
            nc.vector.tensor_scalar(out=z2[:], in0=z2_ps[:], scalar1=b2,
                                    scalar2=0.0, op0=ALU.add, op1=ALU.max)
            m2 = sb.tile([64, 64], F32, tag="m2")
            nc.vector.tensor_scalar(out=m2[:], in0=z2[:], scalar1=0.0,
                                    scalar2=None, op0=ALU.not_equal)

            # ---- Hopfield #1 (PE also squeezes the C_z half of the
            # backward into the latency shadow: C = C_z - C_q * diag(1/s))
            S1_ps = ps.tile([128, 4, 64], F32, tag="S", bufs=1, name="S1")
            for t in range(4):
                nc.tensor.matmul(S1_ps[:, t, :], KTp[:, t * 128:(t + 1) * 128],
                                 z_bf[:], start=True, stop=True)
            # z2 half of the backward (rhs z_bf, no Hopfield dependency)
            g1z_ps = ps.tile([128, 4, 64], F32, tag="gf", bufs=2, name="g1z")
            for t in range(4):
                nc.tensor.matmul(g1z_ps[:, t, :],
                                 w2bp[:, t * 128:(t + 1) * 128],
                                 z_bf[:], start=True, stop=True)
            E1 = sb.tile([128, 4, 64], BF16, tag="E1", name="E1")
            nc.scalar.activation(out=E1[:], in_=S1_ps[:], func=AF.Exp,
                                 bias=0.0, scale=BETA)
            g1zm = sb.tile([128, 4, 64], BF16, tag="g1zm")
            nc.vector.tensor_tensor(out=g1zm[:], in0=g1z_ps[:], in1=M1W[:],
                                    op=ALU.mult)
            Cz_ps = ps.tile([100, 64], F32, tag="misc", bufs=2)
            for t in range(4):
                nc.tensor.matmul(Cz_ps[:], Scomb[:, t, :], g1zm[:, t, :],
                                 start=(t == 0), stop=(t == 3))
            q1_ps = ps.tile([65, 64], F32, tag="q65", bufs=1, name="q1")
            for t in range(4):
                nc.tensor.matmul(q1_ps[:], KV1[:, t, :], E1[:, t, :],
                                 start=(t == 0), stop=(t == 3))

            # ---- q half: rec broadcast via gpsimd, applied at the C level
            rec1 = sb.tile([1, 64], F32, tag="rec1")
            nc.vector.reciprocal(rec1[:], q1_ps[64:65, :])
            qm = sb.tile([64, 64], BF16, tag="qm")
            nc.vector.tensor_tensor(out=qm[:], in0=q1_ps[0:64, :],
                                    in1=m2[:], op=ALU.mult)
            recB_ps = ps.tile([100, 64], F32, tag="qB", bufs=1, name="recB")
            nc.tensor.matmul(recB_ps[:], ones_sb[:], rec1[:],
                             start=True, stop=True)
            recB = sb.tile([100, 64], F32, tag="recB")
            nc.scalar.copy(out=recB[:], in_=recB_ps[:])
            Cz_sb = sb.tile([100, 64], F32, tag="Cz_sb")
            nc.vector.tensor_copy(out=Cz_sb[:], in_=Cz_ps[:])
            g1q_ps = ps.tile([128, 4, 64], F32, tag="gf", bufs=2, name="g1q")
            for t in range(4):
                nc.tensor.matmul(g1q_ps[:, t, :],
                                 w2bp[:, t * 128:(t + 1) * 128],
                                 qm[:], start=True, stop=True)
            g1qm = sb.tile([128, 4, 64], BF16, tag="g1qm")
            nc.vector.tensor_tensor(out=g1qm[:], in0=g1q_ps[:], in1=M1W[:],
                                    op=ALU.mult)
            Cq_ps = ps.tile([100, 64], F32, tag="misc", bufs=2)
            for t in range(4):
                nc.tensor.matmul(Cq_ps[:], Scomb[:, t, :], g1qm[:, t, :],
                                 start=(t == 0), stop=(t == 3))
            t1 = sb.tile([100, 64], F32, tag="t1")
            nc.vector.tensor_tensor(out=t1[:], in0=Cq_ps[:], in1=recB[:],
                                    op=ALU.mult)
            C_bf = sb.tile([100, 64], BF16, tag="C_bf")
            nc.vector.tensor_tensor(out=C_bf[:], in0=Cz_sb[:], in1=t1[:],
                                    op=ALU.subtract)

            # ---- e_min dance (all values exact bf16 copies of C entries,
            # so min/compare are consistent under monotone rounding)
            cand_ps = ps.tile([128, 3, 8, 8], F32, tag="gf", bufs=2,
                              name="cand")
            for cc in range(3):
                nc.tensor.matmul(
                    cand_ps[:, cc, :, :].rearrange("k p q -> k (p q)"),
                    CandM[:, cc, :], C_bf[:], start=True, stop=True)
            cand_sb = sb.tile([128, 3, 8, 8], BF16, tag="cand_sb")
            nc.vector.tensor_copy(out=cand_sb[:], in_=cand_ps[:])
            # shift-aligned candidate stack (zero prefill = min-with-0
            # candidate); innermost-axis min-reduce collapses the 8 classes.
            engs = [nc.vector, nc.vector, nc.vector, nc.vector,
                    nc.gpsimd, nc.gpsimd, nc.gpsimd,
                    nc.scalar, nc.scalar]
            for j, k in enumerate([0, 1, 2, 3, 5, 6, 7, 8, 4]):
                cc, kk = divmod(k, 4)
                dp, dq = k // 3 - 1, k % 3 - 1
                i4lo, i4hi = max(0, dp), min(8, 8 + dp)
                j4lo, j4hi = max(0, dq), min(8, 8 + dq)
                srcap = cand_sb[kk * 32:kk * 32 + 16, cc,
                                i4lo - dp:i4hi - dp,
                                j4lo - dq:j4hi - dq, None]
                dstap = cstk[:, i4lo:i4hi, j4lo:j4hi, j:j + 1]
                if engs[j] is nc.scalar:
                    engs[j].copy(out=dstap, in_=srcap)
                else:
                    engs[j].tensor_copy(out=dstap, in_=srcap)
            nc.vector.tensor_reduce(out=eB[:, 2:10, :], in_=cstk[:],
                                    axis=mybir.AxisListType.X, op=ALU.min)
            eBf = eB[:].rearrange("a b c -> a (b c)")
            eW_ps = ps.tile([100, 64], F32, tag="misc", bufs=2)
            for k in range(9):
                dp, dq = k // 3 - 1, k % 3 - 1
                off = 16 + 8 * dp + dq
                nc.tensor.matmul(eW_ps[:], PermB[:, k * 100:(k + 1) * 100],
                                 eBf[:, off:off + 64],
                                 start=(k == 0), stop=(k == 8))
            maskw = sb.tile([100, 64], F32, tag="maskw")
            nc.vector.tensor_tensor(out=maskw[:], in0=C_bf[:], in1=eW_ps[:],
                                    op=ALU.is_le)

            # ---- masked forward: Xm = X * maskw (broadcast over h), bf16
            Xm = sb.tile([100, 3, 64], BF16, tag="Xm")
            mask_b = bass.AP(tensor=maskw[:].tensor, offset=maskw[:].offset,
                             ap=[[64, 100], [0, 3], [1, 64]])
            nc.vector.tensor_tensor(out=Xm[:], in0=X, in1=mask_b, op=ALU.mult)
            u1_ps = ps.tile([128, 4, 64], F32, tag="gf", bufs=2, name="u1")
            for t in range(4):
                for h in range(3):
                    nc.tensor.matmul(u1_ps[:, t, :], W1big[:, h, t, :],
                                     Xm[:, h, :],
                                     start=(h == 0), stop=(h == 2))
            u1m = sb.tile([128, 4, 64], BF16, tag="u1m")
            nc.vector.tensor_tensor(out=u1m[:], in0=u1_ps[:], in1=M1W[:],
                                    op=ALU.mult)
            zm_ps = ps.tile([64, 64], F32, tag="misc", bufs=2)
            for t in range(4):
                nc.tensor.matmul(zm_ps[:], w2fT[:, t, :], u1m[:, t, :],
                                 start=(t == 0), stop=(t == 3))
            z2m = sb.tile([64, 64], BF16, tag="z2m")
            nc.vector.tensor_tensor(out=z2m[:], in0=zm_ps[:], in1=m2[:],
                                    op=ALU.mult)

            # ---- Hopfield #2 -> output [65, 64]: rows 0:64 unnormalized
            # exp-weighted sums, row 64 the per-pq denominators (host divides)
            q2_ps = _hopfield2(nc, sb, ps, z2m[:], KTp, KV1, "2")
            out_sb = sb.tile([65, 64], F32, tag="out_sb")
            nc.vector.tensor_copy(out=out_sb[:], in_=q2_ps[:])
            nc.sync.dma_start(out=out_t[:], in_=out_sb[:])
    nc.compile()
    return nc


def _get_nc(debug=False):
    key = ("nc", debug)
    if key not in _CACHE:
        _CACHE[key] = _build_nc(debug)
    return _CACHE[key]


# ---------------------------------------------------------------- entry point
def kernel(x, w1, b1, w2, b2, K, Vw, _debug=False):
    x = np.asarray(x, np.float32)
    shared = _host_prep(np.asarray(w1, np.float32), np.asarray(b1, np.float32),
                        np.asarray(w2, np.float32), np.asarray(b2, np.float32),
                        np.asarray(K, np.float32), np.asarray(Vw, np.float32))
    w1f, b1c = shared.pop("_w1f"), shared.pop("_b1")
    b2h = shared.pop("_b2")
    bsz = x.shape[0]
    nc = _get_nc(_debug)
    smpls = [_sample_prep(x[b], w1f, b1c, b2h) for b in range(bsz)]
    in_maps = []
    for core in range(N_CORES):
        m = dict(shared)
        m["cv1"], m["cv2"] = smpls[core] if core < bsz else smpls[0]
        in_maps.append(m)
    res = run_bass_kernel_spmd(nc, in_maps, core_ids=list(range(N_CORES)))
    outs = []
    for b in range(bsz):
        q2 = res.results[b]["out"].astype(np.float32)
        outs.append((q2[0:64] / q2[64:65]).reshape(64, 8, 8))
    out = np.stack(outs).astype(np.float32)
    if _debug:
        return out, res
    return out
